# revision 6
# baseline (speedup 1.0000x reference)
"""Causal multi-head attention block (B=4, S=1024, E=1024, H=16, D=64) on 8 TRN2 cores.

Sharding: data-parallel over batch (4) x tensor-parallel over heads (2 groups of 8).
Core i handles batch i//2, head-group i%2. Each core computes its partial output
projection (row-parallel W_proj); the host sums the two TP partials per batch and
applies the (exact) bias corrections.

Schedule (v2): PE-density-first.
  - Startup: xT streams per-ktile on the sync HWDGE queue while (wq,wk) kt-pairs
    stream on the scalar queue; a kt-major 4-chain group {q0,k0,q1,k1} accumulates
    in all 8 PSUM banks (scoped pool) so the PE saturates as tiles land.
  - Attention: per head-pair p, QK^T chunks ([sk,sq] blocks via row-half pairing,
    concurrent in the two 64-row groups) feed ScalarE exp chunks into a PACKED
    staircase pT buffer (only the causal-valid blocks); diagonal masks are small
    per-block gpsimd multiplies right after their exp chunk. Chain groups q2k2,
    q3k3 and the v chains interleave as PE filler between QK chunks so the exp
    stream never gates the PE.
  - PV: lhsT=v_aug (ones column -> denominator row); normalize = gpsimd
    partition-broadcast of the PSUM denom row + DVE reciprocal + multiply.
  - Tail: pair 3's PV runs per-128-col sq chunks, interleaved with the 8 output
    projection tiles (ACT evacuates PSUM, per-st DMA out on the sync queue).
Host: out[b] = out_partial[2b] + out_partial[2b+1] + (bv_0 Wp_0 + bv_1 Wp_1 + b_proj)
(the v-bias term is exact because softmax rows sum to 1).
"""

import numpy as np
import ml_dtypes

import concourse.bass as bass
import concourse.tile as tile
from concourse import bacc, mybir
from concourse.bass_utils import run_bass_kernel_spmd
from concourse.masks import make_upper_triangular

BF16 = mybir.dt.bfloat16
F32 = mybir.dt.float32

B, S, E = 4, 1024, 1024
H_TOT, D = 16, 64
NCORES = 8
HL = 8            # heads per core
JL = HL * D       # 512 local qkv dim
P = 128
ET = E // P       # 8 k-tiles over embed dim
JT = JL // P      # 4 partition-tiles over local qkv dim

# packed staircase offsets: for sk-tile t, columns sq in [t*P, S) are stored at
# pt[:, POFF[t] + (sq - t*P)]
POFF = [t * S - P * (t * (t - 1) // 2) for t in range(ET)]
PTW = POFF[ET - 1] + (S - (ET - 1) * P)   # 4608 packed cols per head

_NC_CACHE = None


def build_nc():
    nc = bacc.Bacc()

    xT = nc.declare_dram_parameter("xT", [E, S], BF16, isOutput=False)
    wq = nc.declare_dram_parameter("wq", [E, JL], BF16, isOutput=False)
    wk = nc.declare_dram_parameter("wk", [E, JL], BF16, isOutput=False)
    wv = nc.declare_dram_parameter("wv", [E, JL], BF16, isOutput=False)
    wp = nc.declare_dram_parameter("wp", [JL, E], BF16, isOutput=False)
    bq = nc.declare_dram_parameter("bq", [P, JT], F32, isOutput=False)
    bk = nc.declare_dram_parameter("bk", [P, JT], F32, isOutput=False)
    out = nc.declare_dram_parameter("out", [S, E], F32, isOutput=True)

    with tile.TileContext(nc) as tc:
        with (
            tc.tile_pool(name="singles", bufs=1) as singles,
            tc.tile_pool(name="pt", bufs=8) as pt_pool,
            tc.tile_pool(name="bc", bufs=3) as bc_pool,
            tc.tile_pool(name="den", bufs=3) as den_pool,
            tc.tile_pool(name="outst", bufs=2) as out_pool,
        ):
            # ---- static input tiles ----
            xT_sb = singles.tile([P, ET, S], BF16)
            wq_sb = singles.tile([P, ET, JL], BF16)
            wk_sb = singles.tile([P, ET, JL], BF16)
            wv_sb = singles.tile([P, ET, JL], BF16)
            wp_sb = singles.tile([P, JT, E], BF16)
            bq_sb = singles.tile([P, JT], F32)
            bk_sb = singles.tile([P, JT], F32)
            xT_r = xT[:, :].rearrange("(o p) s -> p o s", p=P)
            wq_r = wq[:, :].rearrange("(o p) j -> p o j", p=P)
            wk_r = wk[:, :].rearrange("(o p) j -> p o j", p=P)
            wv_r = wv[:, :].rearrange("(o p) j -> p o j", p=P)
            wp_r = wp[:, :].rearrange("(o p) e -> p o e", p=P)

            # biases on the gpsimd (SWDGE) queue: tiny, cheap trigger
            nc.gpsimd.dma_start(out=bq_sb[:], in_=bq[:, :])
            nc.gpsimd.dma_start(out=bk_sb[:], in_=bk[:, :])
            # (wq,wk) kt-pairs on the scalar HWDGE queue, interleaved so the
            # startup chain group's kt-steps unlock in arrival order
            for c in range(0, ET, 2):
                nc.scalar.dma_start(out=wq_sb[:, c:c + 2], in_=wq_r[:, c:c + 2])
                nc.scalar.dma_start(out=wk_sb[:, c:c + 2], in_=wk_r[:, c:c + 2])
            # xT per-ktile on the sync queue; wv and wp queue AFTER x so the
            # critical x/wq/wk set gets the full DMA bandwidth first
            for kt in range(ET):
                nc.sync.dma_start(out=xT_sb[:, kt], in_=xT_r[:, kt])
            for c in range(0, ET, 2):
                nc.sync.dma_start(out=wv_sb[:, c:c + 2], in_=wv_r[:, c:c + 2])
            nc.sync.dma_start(out=wp_sb[:, 0:2], in_=wp_r[:, 0:2])
            nc.sync.dma_start(out=wp_sb[:, 2:4], in_=wp_r[:, 2:4])

            # causal keep-mask for diagonal PT blocks: 1 where sq >= sk else 0
            mask_sb = singles.tile([P, P], BF16)
            make_upper_triangular(nc, mask_sb[:], val=1.0, diag=True)

            qT_sb = singles.tile([P, JT, S], BF16)   # row j = h*64+d, head-major
            kT_sb = singles.tile([P, JT, S], BF16)
            o2T_sb = singles.tile([P, JT, S], BF16)  # normalized attn out
            vaug_sb = singles.tile([P, ET, HL, D + 1], BF16)  # [sk_p, sk_tile, head, d|ones]
            nc.vector.memset(vaug_sb[:, :, :, D:D + 1], 1.0)

            # ---------- chain groups (q/k projections), kt-major ----------
            def chain_group(jts, pool):
                """Returns (step, finish): step(kt) emits the 4*len(jts) matmuls
                for that ktile; finish() emits the bias-adds. One [P,512] PSUM
                bank per (tag, jt, nb) accumulator."""
                accs = {}
                for jt in jts:
                    for tag in ("q", "k"):
                        for nb in range(2):
                            accs[(tag, jt, nb)] = pool.tile(
                                [P, 512], F32, tag="mm", name=f"c{tag}{jt}{nb}")

                def step(kt):
                    for jt in jts:
                        for tag, w_sb in (("q", wq_sb), ("k", wk_sb)):
                            for nb in range(2):
                                nc.tensor.matmul(
                                    accs[(tag, jt, nb)][:],
                                    lhsT=w_sb[:, kt, jt * P:(jt + 1) * P],
                                    rhs=xT_sb[:, kt, nb * 512:(nb + 1) * 512],
                                    start=(kt == 0), stop=(kt == ET - 1),
                                )

                def finish():
                    for jt in jts:
                        for tag, b_sb, dst in (("q", bq_sb, qT_sb), ("k", bk_sb, kT_sb)):
                            for nb in range(2):
                                nc.vector.tensor_scalar_add(
                                    dst[:, jt, nb * 512:(nb + 1) * 512],
                                    accs[(tag, jt, nb)][:],
                                    b_sb[:, jt:jt + 1],
                                )

                return step, finish

            # ---------- startup: 4-chain kt-major group in 8 PSUM banks ----------
            with tc.tile_pool(name="ps_g0", bufs=8, space="PSUM") as ps_g0:
                g0_step, g0_finish = chain_group([0, 1], ps_g0)
                for kt in range(ET):
                    g0_step(kt)
                g0_finish()

            with (
                tc.tile_pool(name="ps_mm", bufs=4, space="PSUM") as ps_mm,
                tc.tile_pool(name="ps_l", bufs=2, space="PSUM") as ps_l,
                tc.tile_pool(name="ps_o", bufs=2, space="PSUM") as ps_o,
            ):
                pts = {}

                def emit_v_chain(st):
                    ps = ps_mm.tile([P, 512], F32, tag="mm", name=f"v{st}")
                    for kt in range(ET):
                        nc.tensor.matmul(
                            ps[:],
                            lhsT=xT_sb[:, kt, st * P:(st + 1) * P],
                            rhs=wv_sb[:, kt, :],
                            start=(kt == 0), stop=(kt == ET - 1),
                        )
                    nc.vector.tensor_copy(
                        out=vaug_sb[:, st, :, 0:D],
                        in_=ps[:].rearrange("p (h d) -> p h d", h=HL),
                    )

                def emit_pair(p, fillers):
                    """QK^T + exp + per-block diag mask for heads (2p, 2p+1).
                    fillers: list of closures, spread across the t loop."""
                    views = []
                    for hh in (2 * p, 2 * p + 1):
                        jt0, po = hh // 2, (hh % 2) * 64
                        pT = pt_pool.tile([P, PTW], BF16, tag="pt", name=f"pt{hh}")
                        pts[hh] = pT
                        views.append((qT_sb[po:po + 64, jt0, :],
                                      kT_sb[po:po + 64, jt0, :], pT))
                    fi, nf = 0, len(fillers)
                    for t in range(ET):
                        lo = t * P
                        for cb in range(2):
                            c0, c1 = cb * 512, (cb + 1) * 512
                            s0 = max(lo, c0)
                            if s0 >= c1:
                                continue
                            # the two heads' K=64 matmuls sit in disjoint row
                            # halves -> run concurrently on the array
                            psls = []
                            for (qh, kh, _pT) in views:
                                psl = ps_l.tile([P, 512], F32, tag="psl")
                                nc.tensor.matmul(
                                    psl[:, s0 - c0:512],
                                    lhsT=kh[:, lo:lo + P],
                                    rhs=qh[:, s0:c1],
                                    start=True, stop=True,
                                )
                                psls.append(psl)
                            for (_qh, _kh, pT), psl in zip(views, psls):
                                nc.scalar.activation(
                                    out=pT[:, POFF[t] + s0 - lo:POFF[t] + c1 - lo],
                                    in_=psl[:, s0 - c0:512],
                                    func=mybir.ActivationFunctionType.Exp,
                                    scale=0.125,
                                )
                        # diagonal-block causal mask, right after its exp chunk
                        for (_qh, _kh, pT) in views:
                            nc.gpsimd.tensor_mul(
                                out=pT[:, POFF[t]:POFF[t] + P],
                                in0=pT[:, POFF[t]:POFF[t] + P],
                                in1=mask_sb[:],
                            )
                        # interleave filler PE work to hide the exp latency
                        want = (t + 1) * nf // ET
                        while fi < want:
                            fillers[fi]()
                            fi += 1
                    while fi < nf:
                        fillers[fi]()
                        fi += 1

                def norm_chunk(h, pso, r0, r1, c0):
                    """o2T[head rows, c0:c0+(r1-r0)] = pso[:64, r0:r1] / pso[64, r0:r1]"""
                    jt0, po = h // 2, (h % 2) * 64
                    w = r1 - r0
                    den = den_pool.tile([1, 512], F32, tag="den", name=f"dn{h}_{c0}")
                    nc.scalar.copy(out=den[:, 0:w], in_=pso[D:D + 1, r0:r1])
                    nc.vector.reciprocal_approx_fast(out=den[:, 0:w], in_=den[:, 0:w])
                    bcst = bc_pool.tile([64, 512], F32, tag="bc", name=f"bc{h}_{c0}")
                    nc.gpsimd.partition_broadcast(bcst[:, 0:w], den[:, 0:w])
                    nc.vector.tensor_mul(
                        out=o2T_sb[po:po + 64, jt0, c0:c0 + w],
                        in0=pso[:64, r0:r1], in1=bcst[:, 0:w],
                    )

                def emit_pv(h):
                    """full PV + normalize for head h (sq-halves of 512)"""
                    pT = pts[h]
                    for sqb in range(2):
                        c0, c1 = sqb * 512, (sqb + 1) * 512
                        pso = ps_o.tile([P, 512], F32, tag="pso", name=f"pso{h}_{sqb}")
                        ts = [t for t in range(ET) if t * P < c1]
                        for i, t in enumerate(ts):
                            s0 = max(t * P, c0)
                            nc.tensor.matmul(
                                pso[:D + 1, s0 - c0:512],
                                lhsT=vaug_sb[:, t, h, :],
                                rhs=pT[:, POFF[t] + s0 - t * P:POFF[t] + c1 - t * P],
                                start=(i == 0), stop=(i == len(ts) - 1),
                                skip_group_check=True,
                            )
                        norm_chunk(h, pso, 0, 512, c0)

                def emit_pv_fine(h, st, pso):
                    """PV + normalize for head h, one 128-col sq chunk st"""
                    pT = pts[h]
                    c0, r0 = st * P, (st % 4) * P
                    for t in range(st + 1):
                        nc.tensor.matmul(
                            pso[:D + 1, r0:r0 + P],
                            lhsT=vaug_sb[:, t, h, :],
                            rhs=pT[:, POFF[t] + (st - t) * P:POFF[t] + (st - t + 1) * P],
                            start=(t == 0), stop=(t == st),
                            skip_group_check=True,
                        )
                    norm_chunk(h, pso, r0, r0 + P, c0)

                def emit_proj(st):
                    psf = [ps_mm.tile([P, 512], F32, tag="mm", name=f"pj{st}_{eb}")
                           for eb in range(2)]
                    for kt in range(JT):
                        for eb in range(2):
                            nc.tensor.matmul(
                                psf[eb][:],
                                lhsT=o2T_sb[:, kt, st * P:(st + 1) * P],
                                rhs=wp_sb[:, kt, eb * 512:(eb + 1) * 512],
                                start=(kt == 0), stop=(kt == JT - 1),
                            )
                    ob = out_pool.tile([P, 1024], F32)
                    for eb in range(2):
                        nc.scalar.copy(out=ob[:, eb * 512:(eb + 1) * 512], in_=psf[eb][:])
                    nc.sync.dma_start(out=out[st * P:(st + 1) * P, :], in_=ob[:])

                # ---------- master schedule ----------
                g2_step, g2_finish = chain_group([2], ps_mm)
                g2 = [(lambda kt=kt: g2_step(kt)) for kt in range(ET)] + [g2_finish]
                emit_pair(0, g2)

                g3_step, g3_finish = chain_group([3], ps_mm)
                g3 = [(lambda kt=kt: g3_step(kt)) for kt in range(ET)] + [g3_finish]
                emit_pair(1, g3 + [lambda: emit_v_chain(0)])

                emit_pair(2, [lambda st=st: emit_v_chain(st) for st in (1, 2, 3, 4)])
                emit_pair(3, [lambda st=st: emit_v_chain(st) for st in (5, 6, 7)])

                for h in range(0, 6):
                    emit_pv(h)

                # pair 3: per-128-col PV chunks interleaved with the output
                # projection tiles -> tiny serial tail
                psoA = ps_o.tile([P, 512], F32, tag="pso", name="psoA_lo")
                psoB = ps_o.tile([P, 512], F32, tag="pso", name="psoB_lo")
                prev = []
                for st in range(ET):
                    if st == 4:
                        psoA = ps_o.tile([P, 512], F32, tag="pso", name="psoA_hi")
                        psoB = ps_o.tile([P, 512], F32, tag="pso", name="psoB_hi")
                    emit_pv_fine(6, st, psoA)
                    emit_pv_fine(7, st, psoB)
                    prev.append(st)
                    if st >= 1:
                        emit_proj(prev.pop(0))
                for st in prev:
                    emit_proj(st)

    nc.compile()
    return nc


def make_in_maps(x, W_attn, b_attn, W_proj, b_proj):
    bf16 = ml_dtypes.bfloat16
    x = np.asarray(x, dtype=np.float32)
    W_attn = np.asarray(W_attn, dtype=np.float32)
    b_attn = np.asarray(b_attn, dtype=np.float32)
    W_proj = np.asarray(W_proj, dtype=np.float32)
    in_maps = []
    for i in range(NCORES):
        b, g = i // 2, i % 2
        j0 = g * JL
        in_maps.append({
            "xT": np.ascontiguousarray(x[b].T).astype(bf16),
            "wq": W_attn[:, j0:j0 + JL].astype(bf16),
            "wk": W_attn[:, E + j0:E + j0 + JL].astype(bf16),
            "wv": W_attn[:, 2 * E + j0:2 * E + j0 + JL].astype(bf16),
            "wp": W_proj[j0:j0 + JL, :].astype(bf16),
            "bq": np.ascontiguousarray(
                b_attn[j0:j0 + JL].astype(np.float32).reshape(JT, P).T),
            "bk": np.ascontiguousarray(
                b_attn[E + j0:E + j0 + JL].astype(np.float32).reshape(JT, P).T),
        })
    return in_maps


def kernel(x, W_attn, b_attn, W_proj, b_proj):
    global _NC_CACHE
    x = np.asarray(x, dtype=np.float32)
    W_attn = np.asarray(W_attn, dtype=np.float32)
    b_attn = np.asarray(b_attn, dtype=np.float32)
    W_proj = np.asarray(W_proj, dtype=np.float32)
    b_proj = np.asarray(b_proj, dtype=np.float32)

    if _NC_CACHE is None:
        _NC_CACHE = build_nc()
    nc = _NC_CACHE

    in_maps = make_in_maps(x, W_attn, b_attn, W_proj, b_proj)
    res = run_bass_kernel_spmd(nc, in_maps, core_ids=list(range(NCORES)))

    # host unshard: sum the two head-group partials + exact bias corrections
    bias_row = b_proj.copy()
    for g in range(2):
        j0 = g * JL
        bv = b_attn[2 * E + j0:2 * E + j0 + JL].astype(np.float32)
        bias_row += bv @ W_proj[j0:j0 + JL, :].astype(np.float32)

    full = np.empty((B, S, E), np.float32)
    for b in range(B):
        full[b] = (res.results[2 * b]["out"] + res.results[2 * b + 1]["out"]
                   + bias_row[None, :])
    return full


# revision 21
# speedup vs baseline: 1.0400x; 1.0400x over previous
"""Causal multi-head attention block (B=4, S=1024, E=1024, H=16, D=64) on 8 TRN2 cores.

Sharding: data-parallel over batch (4) x tensor-parallel over heads (2 groups of 8).
Core i handles batch i//2, head-group i%2. Each core computes its partial output
projection (row-parallel W_proj); the host sums the two TP partials per batch and
applies the (exact) bias corrections.

Schedule (v2): PE-density-first.
  - Startup: xT streams per-ktile on the sync HWDGE queue while (wq,wk) kt-pairs
    stream on the scalar queue; a kt-major 4-chain group {q0,k0,q1,k1} accumulates
    in all 8 PSUM banks (scoped pool) so the PE saturates as tiles land.
  - Attention: per head-pair p, QK^T chunks ([sk,sq] blocks via row-half pairing,
    concurrent in the two 64-row groups) feed ScalarE exp chunks into a PACKED
    staircase pT buffer (only the causal-valid blocks); diagonal masks are small
    per-block gpsimd multiplies right after their exp chunk. Chain groups q2k2,
    q3k3 and the v chains interleave as PE filler between QK chunks so the exp
    stream never gates the PE.
  - PV: lhsT=v_aug (ones column -> denominator row); normalize = gpsimd
    partition-broadcast of the PSUM denom row + DVE reciprocal + multiply.
  - Tail: pair 3's PV runs per-128-col sq chunks, interleaved with the 8 output
    projection tiles (ACT evacuates PSUM, per-st DMA out on the sync queue).
Host: out[b] = out_partial[2b] + out_partial[2b+1] + (bv_0 Wp_0 + bv_1 Wp_1 + b_proj)
(the v-bias term is exact because softmax rows sum to 1).
"""

import numpy as np
import ml_dtypes

import concourse.bass as bass
import concourse.tile as tile
from concourse import bacc, mybir
from concourse.bass_utils import run_bass_kernel_spmd
from concourse.masks import make_upper_triangular

BF16 = mybir.dt.bfloat16
F32 = mybir.dt.float32

B, S, E = 4, 1024, 1024
H_TOT, D = 16, 64
NCORES = 8
HL = 8            # heads per core
JL = HL * D       # 512 local qkv dim
P = 128
ET = E // P       # 8 k-tiles over embed dim
JT = JL // P      # 4 partition-tiles over local qkv dim

# packed staircase offsets: for sk-tile t, columns sq in [t*P, S) are stored at
# pt[:, POFF[t] + (sq - t*P)]
POFF = [t * S - P * (t * (t - 1) // 2) for t in range(ET)]
PTW = POFF[ET - 1] + (S - (ET - 1) * P)   # 4608 packed cols per head

_NC_CACHE = None


def build_nc():
    nc = bacc.Bacc()

    xT = nc.declare_dram_parameter("xT", [E, S], BF16, isOutput=False)
    wq = nc.declare_dram_parameter("wq", [E, JL], BF16, isOutput=False)
    wk = nc.declare_dram_parameter("wk", [E, JL], BF16, isOutput=False)
    wv = nc.declare_dram_parameter("wv", [E, JL], BF16, isOutput=False)
    wp = nc.declare_dram_parameter("wp", [JL, E], BF16, isOutput=False)
    bq = nc.declare_dram_parameter("bq", [P, JT], F32, isOutput=False)
    bk = nc.declare_dram_parameter("bk", [P, JT], F32, isOutput=False)
    out = nc.declare_dram_parameter("out", [S, E], F32, isOutput=True)

    with tile.TileContext(nc) as tc:
        with (
            tc.tile_pool(name="singles", bufs=1) as singles,
            tc.tile_pool(name="pt", bufs=8) as pt_pool,
            tc.tile_pool(name="bc", bufs=3) as bc_pool,
            tc.tile_pool(name="den", bufs=3) as den_pool,
            tc.tile_pool(name="outst", bufs=2) as out_pool,
        ):
            # ---- static input tiles ----
            xT_sb = singles.tile([P, ET, S], BF16)
            wq_sb = singles.tile([P, ET, JL], BF16)
            wk_sb = singles.tile([P, ET, JL], BF16)
            wv_sb = singles.tile([P, ET, JL], BF16)
            wp_sb = singles.tile([P, JT, E], BF16)
            bq_sb = singles.tile([P, JT], F32)
            bk_sb = singles.tile([P, JT], F32)
            xT_r = xT[:, :].rearrange("(o p) s -> p o s", p=P)
            wq_r = wq[:, :].rearrange("(o p) j -> p o j", p=P)
            wk_r = wk[:, :].rearrange("(o p) j -> p o j", p=P)
            wv_r = wv[:, :].rearrange("(o p) j -> p o j", p=P)
            wp_r = wp[:, :].rearrange("(o p) e -> p o e", p=P)

            # biases on the gpsimd (SWDGE) queue: tiny, cheap trigger
            nc.gpsimd.dma_start(out=bq_sb[:], in_=bq[:, :])
            nc.gpsimd.dma_start(out=bk_sb[:], in_=bk[:, :])
            # (wq,wk) kt-pairs on the scalar HWDGE queue, interleaved so the
            # startup chain group's kt-steps unlock in arrival order
            for c in range(0, ET, 2):
                nc.scalar.dma_start(out=wq_sb[:, c:c + 2], in_=wq_r[:, c:c + 2])
                nc.scalar.dma_start(out=wk_sb[:, c:c + 2], in_=wk_r[:, c:c + 2])
            # xT per-ktile on the sync queue; wv and wp queue AFTER x so the
            # critical x/wq/wk set gets the full DMA bandwidth first
            for kt in range(ET):
                nc.sync.dma_start(out=xT_sb[:, kt], in_=xT_r[:, kt])
            for c in range(0, ET, 2):
                nc.sync.dma_start(out=wv_sb[:, c:c + 2], in_=wv_r[:, c:c + 2])
            nc.sync.dma_start(out=wp_sb[:, 0:2], in_=wp_r[:, 0:2])
            nc.sync.dma_start(out=wp_sb[:, 2:4], in_=wp_r[:, 2:4])

            # causal keep-mask for diagonal PT blocks: 1 where sq >= sk else 0
            mask_sb = singles.tile([P, P], BF16)
            make_upper_triangular(nc, mask_sb[:], val=1.0, diag=True)

            qT_sb = singles.tile([P, JT, S], BF16)   # row j = h*64+d, head-major
            kT_sb = singles.tile([P, JT, S], BF16)
            o2T_sb = singles.tile([P, JT, S], BF16)  # normalized attn out
            vaug_sb = singles.tile([P, ET, HL, D + 1], BF16)  # [sk_p, sk_tile, head, d|ones]
            nc.vector.memset(vaug_sb[:, :, :, D:D + 1], 1.0)

            # ---------- chain groups (q/k projections), kt-major ----------
            def chain_group(jts, pool):
                """Returns (step, finish): step(kt) emits the 4*len(jts) matmuls
                for that ktile; finish() emits the bias-adds. One [P,512] PSUM
                bank per (tag, jt, nb) accumulator."""
                accs = {}
                for jt in jts:
                    for tag in ("q", "k"):
                        for nb in range(2):
                            accs[(tag, jt, nb)] = pool.tile(
                                [P, 512], F32, tag="mm", name=f"c{tag}{jt}{nb}")

                def step(kt):
                    for jt in jts:
                        for tag, w_sb in (("q", wq_sb), ("k", wk_sb)):
                            for nb in range(2):
                                nc.tensor.matmul(
                                    accs[(tag, jt, nb)][:],
                                    lhsT=w_sb[:, kt, jt * P:(jt + 1) * P],
                                    rhs=xT_sb[:, kt, nb * 512:(nb + 1) * 512],
                                    start=(kt == 0), stop=(kt == ET - 1),
                                )

                def finish():
                    for jt in jts:
                        for tag, b_sb, dst in (("q", bq_sb, qT_sb), ("k", bk_sb, kT_sb)):
                            for nb in range(2):
                                nc.vector.tensor_scalar_add(
                                    dst[:, jt, nb * 512:(nb + 1) * 512],
                                    accs[(tag, jt, nb)][:],
                                    b_sb[:, jt:jt + 1],
                                )

                return step, finish

            # ---------- startup: 4-chain kt-major group in 8 PSUM banks ----------
            with tc.tile_pool(name="ps_g0", bufs=8, space="PSUM") as ps_g0:
                g0_step, g0_finish = chain_group([0, 1], ps_g0)
                for kt in range(ET):
                    g0_step(kt)
                g0_finish()

            with (
                tc.tile_pool(name="ps_mm", bufs=2, space="PSUM") as ps_mm,
                tc.tile_pool(name="ps_l", bufs=2, space="PSUM") as ps_l,
                tc.tile_pool(name="ps_o", bufs=2, space="PSUM") as ps_o,
            ):
                pts = {}

                def emit_chain(jt, tag):
                    """one q- or k-projection chain (2 PSUM slots, serial kt)"""
                    w_sb, b_sb, dst = ((wq_sb, bq_sb, qT_sb) if tag == "q"
                                       else (wk_sb, bk_sb, kT_sb))
                    accs = [ps_mm.tile([P, 512], F32, tag="mm", name=f"c{tag}{jt}{nb}")
                            for nb in range(2)]
                    for kt in range(ET):
                        for nb in range(2):
                            nc.tensor.matmul(
                                accs[nb][:],
                                lhsT=w_sb[:, kt, jt * P:(jt + 1) * P],
                                rhs=xT_sb[:, kt, nb * 512:(nb + 1) * 512],
                                start=(kt == 0), stop=(kt == ET - 1),
                            )
                    for nb in range(2):
                        nc.vector.tensor_scalar_add(
                            dst[:, jt, nb * 512:(nb + 1) * 512],
                            accs[nb][:], b_sb[:, jt:jt + 1],
                        )

                def emit_v_chain(st):
                    ps = ps_mm.tile([P, 512], F32, tag="mm", name=f"v{st}")
                    for kt in range(ET):
                        nc.tensor.matmul(
                            ps[:],
                            lhsT=xT_sb[:, kt, st * P:(st + 1) * P],
                            rhs=wv_sb[:, kt, :],
                            start=(kt == 0), stop=(kt == ET - 1),
                        )
                    nc.vector.tensor_copy(
                        out=vaug_sb[:, st, :, 0:D],
                        in_=ps[:].rearrange("p (h d) -> p h d", h=HL),
                    )

                def emit_pair(p, fillers):
                    """QK^T + exp + per-block diag mask for heads (2p, 2p+1).
                    fillers: list of closures, spread across the t loop."""
                    views = []
                    for hh in (2 * p, 2 * p + 1):
                        jt0, po = hh // 2, (hh % 2) * 64
                        pT = pt_pool.tile([P, PTW], BF16, tag="pt", name=f"pt{hh}")
                        pts[hh] = pT
                        views.append((qT_sb[po:po + 64, jt0, :],
                                      kT_sb[po:po + 64, jt0, :], pT))
                    fi, nf = 0, len(fillers)
                    for t in range(ET):
                        lo = t * P
                        # psl chunks at ABSOLUTE sq offsets: each <=512-col
                        # matmul stays inside one 2KB PSUM bank
                        psls = [ps_l.tile([P, 1024], F32, tag="psl",
                                          name=f"psl{p}_{t}_{vi}")
                                for vi in range(len(views))]
                        for cb in range(2):
                            c0, c1 = cb * 512, (cb + 1) * 512
                            s0 = max(lo, c0)
                            if s0 >= c1:
                                continue
                            # the two heads' K=64 matmuls sit in disjoint row
                            # halves -> run concurrently on the array
                            for (qh, kh, _pT), psl in zip(views, psls):
                                nc.tensor.matmul(
                                    psl[:, s0:c1],
                                    lhsT=kh[:, lo:lo + P],
                                    rhs=qh[:, s0:c1],
                                    start=True, stop=True,
                                )
                        for (_qh, _kh, pT), psl in zip(views, psls):
                            nc.scalar.activation(
                                out=pT[:, POFF[t]:POFF[t] + S - lo],
                                in_=psl[:, lo:S],
                                func=mybir.ActivationFunctionType.Exp,
                                scale=0.125,
                            )
                        # diagonal-block causal mask, right after its exp
                        for (_qh, _kh, pT) in views:
                            nc.gpsimd.tensor_mul(
                                out=pT[:, POFF[t]:POFF[t] + P],
                                in0=pT[:, POFF[t]:POFF[t] + P],
                                in1=mask_sb[:],
                            )
                        # interleave filler PE work to hide the exp latency
                        want = (t + 1) * nf // ET
                        while fi < want:
                            fillers[fi]()
                            fi += 1
                    while fi < nf:
                        fillers[fi]()
                        fi += 1

                def norm_chunk(h, pso, r0, r1, c0):
                    """o2T[head rows, c0:c0+(r1-r0)] = pso[:64, r0:r1] / pso[64, r0:r1]"""
                    jt0, po = h // 2, (h % 2) * 64
                    w = r1 - r0
                    den = den_pool.tile([1, 512], F32, tag="den", name=f"dn{h}_{c0}")
                    # custom-DVE reciprocal needs SBUF input at partition 0: stage first
                    nc.vector.tensor_copy(out=den[:, 0:w], in_=pso[D:D + 1, r0:r1])
                    nc.vector.reciprocal_approx_fast(out=den[:, 0:w], in_=den[:, 0:w])
                    bcst = bc_pool.tile([64, 512], F32, tag="bc", name=f"bc{h}_{c0}")
                    nc.gpsimd.partition_broadcast(bcst[:, 0:w], den[:, 0:w])
                    nc.vector.tensor_mul(
                        out=o2T_sb[po:po + 64, jt0, c0:c0 + w],
                        in0=pso[:D, r0:r1], in1=bcst[:, 0:w],
                    )

                def emit_pv(h):
                    """full PV + normalize for head h (sq-halves of 512)"""
                    pT = pts[h]
                    for sqb in range(2):
                        c0, c1 = sqb * 512, (sqb + 1) * 512
                        pso = ps_o.tile([P, 512], F32, tag="pso", name=f"pso{h}_{sqb}")
                        ts = [t for t in range(ET) if t * P < c1]
                        for i, t in enumerate(ts):
                            s0 = max(t * P, c0)
                            nc.tensor.matmul(
                                pso[:D + 1, s0 - c0:512],
                                lhsT=vaug_sb[:, t, h, :],
                                rhs=pT[:, POFF[t] + s0 - t * P:POFF[t] + c1 - t * P],
                                start=(i == 0), stop=(i == len(ts) - 1),
                                skip_group_check=True,
                            )
                        norm_chunk(h, pso, 0, 512, c0)

                def emit_pv_fine(h, st, pso):
                    """PV + normalize for head h, one 128-col sq chunk st"""
                    pT = pts[h]
                    c0, r0 = st * P, (st % 4) * P
                    for t in range(st + 1):
                        nc.tensor.matmul(
                            pso[:D + 1, r0:r0 + P],
                            lhsT=vaug_sb[:, t, h, :],
                            rhs=pT[:, POFF[t] + (st - t) * P:POFF[t] + (st - t + 1) * P],
                            start=(t == 0), stop=(t == st),
                            skip_group_check=True,
                        )
                    norm_chunk(h, pso, r0, r0 + P, c0)

                def emit_proj(st):
                    psf = [ps_mm.tile([P, 512], F32, tag="mm", name=f"pj{st}{eb}")
                           for eb in range(2)]
                    for kt in range(JT):
                        for eb in range(2):
                            nc.tensor.matmul(
                                psf[eb][:],
                                lhsT=o2T_sb[:, kt, st * P:(st + 1) * P],
                                rhs=wp_sb[:, kt, eb * 512:(eb + 1) * 512],
                                start=(kt == 0), stop=(kt == JT - 1),
                            )
                    ob = out_pool.tile([P, 1024], F32)
                    for eb in range(2):
                        nc.scalar.copy(out=ob[:, eb * 512:(eb + 1) * 512], in_=psf[eb][:])
                    nc.sync.dma_start(out=out[st * P:(st + 1) * P, :], in_=ob[:])

                # ---------- master schedule ----------
                emit_pair(0, [lambda: emit_chain(2, "q"), lambda: emit_chain(2, "k")])
                emit_pair(1, [lambda: emit_chain(3, "q"), lambda: emit_chain(3, "k"),
                              lambda: emit_v_chain(0)])
                emit_pair(2, [lambda st=st: emit_v_chain(st) for st in (1, 2, 3, 4)])
                emit_pair(3, [lambda st=st: emit_v_chain(st) for st in (5, 6, 7)])

                for h in range(0, 6):
                    emit_pv(h)

                # pair 3: per-128-col PV chunks interleaved with the output
                # projection tiles -> tiny serial tail
                psoA = ps_o.tile([P, 512], F32, tag="pso", name="psoA_lo")
                psoB = ps_o.tile([P, 512], F32, tag="pso", name="psoB_lo")
                prev = []
                for st in range(ET):
                    if st == 4:
                        psoA = ps_o.tile([P, 512], F32, tag="pso", name="psoA_hi")
                        psoB = ps_o.tile([P, 512], F32, tag="pso", name="psoB_hi")
                    emit_pv_fine(6, st, psoA)
                    emit_pv_fine(7, st, psoB)
                    prev.append(st)
                    if st >= 1:
                        emit_proj(prev.pop(0))
                for st in prev:
                    emit_proj(st)

    nc.compile()
    return nc


def make_in_maps(x, W_attn, b_attn, W_proj, b_proj):
    bf16 = ml_dtypes.bfloat16
    x = np.asarray(x, dtype=np.float32)
    W_attn = np.asarray(W_attn, dtype=np.float32)
    b_attn = np.asarray(b_attn, dtype=np.float32)
    W_proj = np.asarray(W_proj, dtype=np.float32)
    in_maps = []
    for i in range(NCORES):
        b, g = i // 2, i % 2
        j0 = g * JL
        in_maps.append({
            "xT": np.ascontiguousarray(x[b].T).astype(bf16),
            "wq": W_attn[:, j0:j0 + JL].astype(bf16),
            "wk": W_attn[:, E + j0:E + j0 + JL].astype(bf16),
            "wv": W_attn[:, 2 * E + j0:2 * E + j0 + JL].astype(bf16),
            "wp": W_proj[j0:j0 + JL, :].astype(bf16),
            "bq": np.ascontiguousarray(
                b_attn[j0:j0 + JL].astype(np.float32).reshape(JT, P).T),
            "bk": np.ascontiguousarray(
                b_attn[E + j0:E + j0 + JL].astype(np.float32).reshape(JT, P).T),
        })
    return in_maps


def kernel(x, W_attn, b_attn, W_proj, b_proj):
    global _NC_CACHE
    x = np.asarray(x, dtype=np.float32)
    W_attn = np.asarray(W_attn, dtype=np.float32)
    b_attn = np.asarray(b_attn, dtype=np.float32)
    W_proj = np.asarray(W_proj, dtype=np.float32)
    b_proj = np.asarray(b_proj, dtype=np.float32)

    if _NC_CACHE is None:
        _NC_CACHE = build_nc()
    nc = _NC_CACHE

    in_maps = make_in_maps(x, W_attn, b_attn, W_proj, b_proj)
    res = run_bass_kernel_spmd(nc, in_maps, core_ids=list(range(NCORES)))

    # host unshard: sum the two head-group partials + exact bias corrections
    bias_row = b_proj.copy()
    for g in range(2):
        j0 = g * JL
        bv = b_attn[2 * E + j0:2 * E + j0 + JL].astype(np.float32)
        bias_row += bv @ W_proj[j0:j0 + JL, :].astype(np.float32)

    full = np.empty((B, S, E), np.float32)
    for b in range(B):
        full[b] = (res.results[2 * b]["out"] + res.results[2 * b + 1]["out"]
                   + bias_row[None, :])
    return full


# revision 32
# speedup vs baseline: 1.1653x; 1.1205x over previous
"""Causal multi-head attention block (B=4, S=1024, E=1024, H=16, D=64) on 8 TRN2 cores.

Sharding: data-parallel over batch (4) x tensor-parallel over heads (2 groups of 8).
Core i handles batch i//2, head-group i%2. Each core computes its partial output
projection (row-parallel W_proj); the host sums the two TP partials per batch and
applies the (exact) bias corrections.

Schedule (v2): PE-density-first.
  - Startup: xT streams per-ktile on the sync HWDGE queue while (wq,wk) kt-pairs
    stream on the scalar queue; a kt-major 4-chain group {q0,k0,q1,k1} accumulates
    in all 8 PSUM banks (scoped pool) so the PE saturates as tiles land.
  - Attention: per head-pair p, QK^T chunks ([sk,sq] blocks via row-half pairing,
    concurrent in the two 64-row groups) feed ScalarE exp chunks into a PACKED
    staircase pT buffer (only the causal-valid blocks); diagonal masks are small
    per-block gpsimd multiplies right after their exp chunk. Chain groups q2k2,
    q3k3 and the v chains interleave as PE filler between QK chunks so the exp
    stream never gates the PE.
  - PV: lhsT=v_aug (ones column -> denominator row); normalize = gpsimd
    partition-broadcast of the PSUM denom row + DVE reciprocal + multiply.
  - Tail: pair 3's PV runs per-128-col sq chunks, interleaved with the 8 output
    projection tiles (ACT evacuates PSUM, per-st DMA out on the sync queue).
Host: out[b] = out_partial[2b] + out_partial[2b+1] + (bv_0 Wp_0 + bv_1 Wp_1 + b_proj)
(the v-bias term is exact because softmax rows sum to 1).
"""

import numpy as np
import ml_dtypes

import concourse.bass as bass
import concourse.tile as tile
from concourse import bacc, mybir
from concourse.bass_utils import run_bass_kernel_spmd
from concourse.masks import make_upper_triangular

BF16 = mybir.dt.bfloat16
F32 = mybir.dt.float32

B, S, E = 4, 1024, 1024
H_TOT, D = 16, 64
NCORES = 8
HL = 8            # heads per core
JL = HL * D       # 512 local qkv dim
P = 128
ET = E // P       # 8 k-tiles over embed dim
JT = JL // P      # 4 partition-tiles over local qkv dim

# packed staircase offsets: for sk-tile t, columns sq in [t*P, S) are stored at
# pt[:, POFF[t] + (sq - t*P)]
POFF = [t * S - P * (t * (t - 1) // 2) for t in range(ET)]
PTW = POFF[ET - 1] + (S - (ET - 1) * P)   # 4608 packed cols per head

_NC_CACHE = None


def build_nc():
    nc = bacc.Bacc()

    xT = nc.declare_dram_parameter("xT", [E, S], BF16, isOutput=False)
    wq = nc.declare_dram_parameter("wq", [E, JL], BF16, isOutput=False)
    wk = nc.declare_dram_parameter("wk", [E, JL], BF16, isOutput=False)
    wv = nc.declare_dram_parameter("wv", [E, JL], BF16, isOutput=False)
    wp = nc.declare_dram_parameter("wp", [JL, E], BF16, isOutput=False)
    bq = nc.declare_dram_parameter("bq", [P, JT], F32, isOutput=False)
    bk = nc.declare_dram_parameter("bk", [P, JT], F32, isOutput=False)
    out = nc.declare_dram_parameter("out", [S, E], F32, isOutput=True)

    with tile.TileContext(nc) as tc:
        with (
            tc.tile_pool(name="singles", bufs=1) as singles,
            tc.tile_pool(name="pt", bufs=8) as pt_pool,
            tc.tile_pool(name="bc", bufs=3) as bc_pool,
            tc.tile_pool(name="den", bufs=3) as den_pool,
            tc.tile_pool(name="outst", bufs=2) as out_pool,
        ):
            # ---- static input tiles ----
            xT_sb = singles.tile([P, ET, S], BF16)
            wq_sb = singles.tile([P, ET, JL], BF16)
            wk_sb = singles.tile([P, ET, JL], BF16)
            wv_sb = singles.tile([P, ET, JL], BF16)
            wp_sb = singles.tile([P, JT, E], BF16)
            bq_sb = singles.tile([P, JT], F32)
            bk_sb = singles.tile([P, JT], F32)
            xT_r = xT[:, :].rearrange("(o p) s -> p o s", p=P)
            wq_r = wq[:, :].rearrange("(o p) j -> p o j", p=P)
            wk_r = wk[:, :].rearrange("(o p) j -> p o j", p=P)
            wv_r = wv[:, :].rearrange("(o p) j -> p o j", p=P)
            wp_r = wp[:, :].rearrange("(o p) e -> p o e", p=P)

            # biases on the gpsimd (SWDGE) queue: tiny, cheap trigger
            nc.gpsimd.dma_start(out=bq_sb[:], in_=bq[:, :])
            nc.gpsimd.dma_start(out=bk_sb[:], in_=bk[:, :])
            # (wq,wk) kt-pairs on the scalar HWDGE queue, interleaved so the
            # startup chain group's kt-steps unlock in arrival order
            for c in range(0, ET, 2):
                nc.scalar.dma_start(out=wq_sb[:, c:c + 2], in_=wq_r[:, c:c + 2])
                nc.scalar.dma_start(out=wk_sb[:, c:c + 2], in_=wk_r[:, c:c + 2])
            # xT per-ktile on the sync queue; wv and wp queue AFTER x so the
            # critical x/wq/wk set gets the full DMA bandwidth first
            for kt in range(ET):
                nc.sync.dma_start(out=xT_sb[:, kt], in_=xT_r[:, kt])
            for c in range(0, ET, 2):
                nc.sync.dma_start(out=wv_sb[:, c:c + 2], in_=wv_r[:, c:c + 2])
            nc.sync.dma_start(out=wp_sb[:, 0:2], in_=wp_r[:, 0:2])
            nc.sync.dma_start(out=wp_sb[:, 2:4], in_=wp_r[:, 2:4])

            # causal keep-mask for diagonal PT blocks: 1 where sq >= sk else 0
            mask_sb = singles.tile([P, P], BF16)
            make_upper_triangular(nc, mask_sb[:], val=1.0, diag=True)

            qT_sb = singles.tile([P, JT, S], BF16)   # row j = h*64+d, head-major
            kT_sb = singles.tile([P, JT, S], BF16)
            o2T_sb = singles.tile([P, JT, S], BF16)  # normalized attn out
            vaug_sb = singles.tile([P, ET, HL, D + 1], BF16)  # [sk_p, sk_tile, head, d|ones]
            nc.vector.memset(vaug_sb[:, :, :, D:D + 1], 1.0)

            # ---------- chain groups (q/k projections), kt-major ----------
            def chain_group(jts, pool, split=False):
                """Returns (step, finish): step(kt) emits the 4*len(jts) matmuls
                for that ktile; finish(jt) the bias-adds (q on DVE, k on gpsimd
                so the two biases drain in parallel)."""
                accs = {}
                for jt in jts:
                    for tag in ("q", "k"):
                        for nb in range(2):
                            accs[(tag, jt, nb)] = pool.tile(
                                [P, 512], F32, tag="mm", name=f"c{tag}{jt}{nb}")

                def step(kt):
                    for jt in jts:
                        for tag, w_sb in (("q", wq_sb), ("k", wk_sb)):
                            for nb in range(2):
                                nc.tensor.matmul(
                                    accs[(tag, jt, nb)][:],
                                    lhsT=w_sb[:, kt, jt * P:(jt + 1) * P],
                                    rhs=xT_sb[:, kt, nb * 512:(nb + 1) * 512],
                                    start=(kt == 0), stop=(kt == ET - 1),
                                )

                def finish(jt):
                    # q-bias on DVE, k-bias on ACT (idle pre-exp) in parallel
                    for nb in range(2):
                        nc.vector.tensor_scalar_add(
                            qT_sb[:, jt, nb * 512:(nb + 1) * 512],
                            accs[("q", jt, nb)][:], bq_sb[:, jt:jt + 1],
                        )
                    for nb in range(2):
                        nc.scalar.activation(
                            out=kT_sb[:, jt, nb * 512:(nb + 1) * 512],
                            in_=accs[("k", jt, nb)][:],
                            func=mybir.ActivationFunctionType.Identity,
                            bias=bk_sb[:, jt:jt + 1],
                        )

                return step, finish

            # ---------- HAM warm-up: dummy matmuls during the DMA-wait window
            # (PE runs at 1.2 GHz until ~3.4us of continuous activity) ----------
            warm_sb = singles.tile([P, 512], BF16)
            nc.vector.memset(warm_sb[:], 0.5)
            with tc.tile_pool(name="ps_w", bufs=1, space="PSUM") as ps_w:
                wps = ps_w.tile([P, 512], F32, tag="w", name="warm")
                NWARM = 12
                for i in range(NWARM):
                    nc.tensor.matmul(
                        wps[:], lhsT=warm_sb[:, 0:P], rhs=warm_sb[:],
                        start=(i == 0), stop=(i == NWARM - 1),
                    )
                nc.vector.tensor_copy(out=warm_sb[:, 0:1], in_=wps[:, 0:1])

            # ---------- startup: 4-chain kt-major group in 8 PSUM banks ----------
            with tc.tile_pool(name="ps_g0", bufs=8, space="PSUM") as ps_g0:
                g0_step, g0_finish = chain_group([0, 1], ps_g0)
                for kt in range(ET):
                    g0_step(kt)
                g0_finish(0)
                g0_finish(1)

            with (
                tc.tile_pool(name="ps_mm", bufs=2, space="PSUM") as ps_mm,
                tc.tile_pool(name="ps_l", bufs=2, space="PSUM") as ps_l,
                tc.tile_pool(name="ps_o", bufs=2, space="PSUM") as ps_o,
            ):
                pts = {}

                def emit_chain(jt, tag):
                    """one q- or k-projection chain (2 PSUM slots, serial kt)"""
                    w_sb, b_sb, dst = ((wq_sb, bq_sb, qT_sb) if tag == "q"
                                       else (wk_sb, bk_sb, kT_sb))
                    accs = [ps_mm.tile([P, 512], F32, tag="mm", name=f"c{tag}{jt}{nb}")
                            for nb in range(2)]
                    for kt in range(ET):
                        for nb in range(2):
                            nc.tensor.matmul(
                                accs[nb][:],
                                lhsT=w_sb[:, kt, jt * P:(jt + 1) * P],
                                rhs=xT_sb[:, kt, nb * 512:(nb + 1) * 512],
                                start=(kt == 0), stop=(kt == ET - 1),
                            )
                    for nb in range(2):
                        nc.vector.tensor_scalar_add(
                            dst[:, jt, nb * 512:(nb + 1) * 512],
                            accs[nb][:], b_sb[:, jt:jt + 1],
                        )

                def emit_v_chain(st):
                    ps = ps_mm.tile([P, 512], F32, tag="mm", name=f"v{st}")
                    for kt in range(ET):
                        nc.tensor.matmul(
                            ps[:],
                            lhsT=xT_sb[:, kt, st * P:(st + 1) * P],
                            rhs=wv_sb[:, kt, :],
                            start=(kt == 0), stop=(kt == ET - 1),
                        )
                    nc.vector.tensor_copy(
                        out=vaug_sb[:, st, :, 0:D],
                        in_=ps[:].rearrange("p (h d) -> p h d", h=HL),
                    )

                def emit_pair(p, fillers):
                    """QK^T + exp + per-block diag mask for heads (2p, 2p+1).
                    fillers: list of closures, spread across the t loop."""
                    views = []
                    for hh in (2 * p, 2 * p + 1):
                        jt0, po = hh // 2, (hh % 2) * 64
                        pT = pt_pool.tile([P, PTW], BF16, tag="pt", name=f"pt{hh}")
                        pts[hh] = pT
                        views.append((qT_sb[po:po + 64, jt0, :],
                                      kT_sb[po:po + 64, jt0, :], pT))
                    fi, nf = 0, len(fillers)
                    for t in range(ET):
                        lo = t * P
                        # psl chunks at ABSOLUTE sq offsets: each <=512-col
                        # matmul stays inside one 2KB PSUM bank
                        psls = [ps_l.tile([P, 1024], F32, tag="psl",
                                          name=f"psl{p}_{t}_{vi}")
                                for vi in range(len(views))]
                        for cb in range(2):
                            c0, c1 = cb * 512, (cb + 1) * 512
                            s0 = max(lo, c0)
                            if s0 >= c1:
                                continue
                            # the two heads' K=64 matmuls sit in disjoint row
                            # halves -> run concurrently on the array
                            for (qh, kh, _pT), psl in zip(views, psls):
                                nc.tensor.matmul(
                                    psl[:, s0:c1],
                                    lhsT=kh[:, lo:lo + P],
                                    rhs=qh[:, s0:c1],
                                    start=True, stop=True,
                                )
                        for (_qh, _kh, pT), psl in zip(views, psls):
                            nc.scalar.activation(
                                out=pT[:, POFF[t]:POFF[t] + S - lo],
                                in_=psl[:, lo:S],
                                func=mybir.ActivationFunctionType.Exp,
                                scale=0.125,
                            )
                        # diagonal-block causal mask, right after its exp
                        # (head A on DVE, head B on gpsimd: balances both queues)
                        for vi, (_qh, _kh, pT) in enumerate(views):
                            eng = nc.vector if vi == 0 else nc.gpsimd
                            eng.tensor_mul(
                                out=pT[:, POFF[t]:POFF[t] + P],
                                in0=pT[:, POFF[t]:POFF[t] + P],
                                in1=mask_sb[:],
                            )
                        # interleave filler PE work to hide the exp latency
                        want = (t + 1) * nf // ET
                        while fi < want:
                            fillers[fi]()
                            fi += 1
                    while fi < nf:
                        fillers[fi]()
                        fi += 1

                def norm_chunk(h, pso, r0, r1, c0):
                    """o2T[head rows, c0:c0+(r1-r0)] = pso[:64, r0:r1] / pso[64, r0:r1]"""
                    jt0, po = h // 2, (h % 2) * 64
                    w = r1 - r0
                    den = den_pool.tile([1, 512], F32, tag="den", name=f"dn{h}_{c0}")
                    # custom-DVE reciprocal needs SBUF input at partition 0: stage first
                    nc.vector.tensor_copy(out=den[:, 0:w], in_=pso[D:D + 1, r0:r1])
                    nc.vector.reciprocal_approx_fast(out=den[:, 0:w], in_=den[:, 0:w])
                    bcst = bc_pool.tile([64, 512], F32, tag="bc", name=f"bc{h}_{c0}")
                    nc.gpsimd.partition_broadcast(bcst[:, 0:w], den[:, 0:w])
                    nc.vector.tensor_mul(
                        out=o2T_sb[po:po + 64, jt0, c0:c0 + w],
                        in0=pso[:D, r0:r1], in1=bcst[:, 0:w],
                    )

                def emit_pv_sqb(h, sqb):
                    """PV + normalize for head h, one 512-col sq half"""
                    pT = pts[h]
                    c0, c1 = sqb * 512, (sqb + 1) * 512
                    pso = ps_o.tile([P, 512], F32, tag="pso", name=f"pso{h}_{sqb}")
                    ts = [t for t in range(ET) if t * P < c1]
                    for i, t in enumerate(ts):
                        s0 = max(t * P, c0)
                        nc.tensor.matmul(
                            pso[:D + 1, s0 - c0:512],
                            lhsT=vaug_sb[:, t, h, :],
                            rhs=pT[:, POFF[t] + s0 - t * P:POFF[t] + c1 - t * P],
                            start=(i == 0), stop=(i == len(ts) - 1),
                            skip_group_check=True,
                        )
                    norm_chunk(h, pso, 0, 512, c0)

                def emit_pv_fine(h, st, pso):
                    """PV + normalize for head h, one 128-col sq chunk st"""
                    pT = pts[h]
                    c0, r0 = st * P, (st % 4) * P
                    for t in range(st + 1):
                        nc.tensor.matmul(
                            pso[:D + 1, r0:r0 + P],
                            lhsT=vaug_sb[:, t, h, :],
                            rhs=pT[:, POFF[t] + (st - t) * P:POFF[t] + (st - t + 1) * P],
                            start=(t == 0), stop=(t == st),
                            skip_group_check=True,
                        )
                    norm_chunk(h, pso, r0, r0 + P, c0)

                def emit_proj(st):
                    # [P,1024] accumulator in the (drained) wide psl slots:
                    # eb halves live in separate banks, single ACT evacuation
                    psf = ps_l.tile([P, 1024], F32, tag="psl", name=f"pj{st}")
                    for kt in range(JT):
                        for eb in range(2):
                            nc.tensor.matmul(
                                psf[:, eb * 512:(eb + 1) * 512],
                                lhsT=o2T_sb[:, kt, st * P:(st + 1) * P],
                                rhs=wp_sb[:, kt, eb * 512:(eb + 1) * 512],
                                start=(kt == 0), stop=(kt == JT - 1),
                                skip_group_check=True,
                            )
                    ob = out_pool.tile([P, 1024], F32)
                    nc.scalar.copy(out=ob[:], in_=psf[:])
                    nc.sync.dma_start(out=out[st * P:(st + 1) * P, :], in_=ob[:])

                # ---------- master schedule ----------
                # PV sq-halves ride inside the (exp-paced) pair slots as PE
                # filler; their norm chains overlap the exp stream.
                emit_pair(0, [lambda: emit_chain(2, "q"), lambda: emit_chain(2, "k")])
                emit_pair(1, [lambda: emit_chain(3, "q"), lambda: emit_chain(3, "k"),
                              lambda: emit_v_chain(0), lambda: emit_v_chain(1)])
                emit_pair(2, [lambda st=st: emit_v_chain(st) for st in (2, 3, 4)]
                          + [lambda: emit_pv_sqb(0, 0), lambda: emit_pv_sqb(1, 0)])
                emit_pair(3, [lambda st=st: emit_v_chain(st) for st in (5, 6, 7)]
                          + [lambda: emit_pv_sqb(2, 0), lambda: emit_pv_sqb(3, 0),
                             lambda: emit_pv_sqb(0, 1), lambda: emit_pv_sqb(1, 1)])

                for h, sqb in ((4, 0), (5, 0), (2, 1), (3, 1), (4, 1), (5, 1)):
                    emit_pv_sqb(h, sqb)

                # pair 3: per-128-col PV chunks (fine psos in the drained
                # chain banks) interleaved with the output projection tiles
                psoA = ps_mm.tile([P, 512], F32, tag="mm", name="psoA_lo")
                psoB = ps_mm.tile([P, 512], F32, tag="mm", name="psoB_lo")
                prev = []
                for st in range(ET):
                    if st == 4:
                        psoA = ps_mm.tile([P, 512], F32, tag="mm", name="psoA_hi")
                        psoB = ps_mm.tile([P, 512], F32, tag="mm", name="psoB_hi")
                    emit_pv_fine(6, st, psoA)
                    emit_pv_fine(7, st, psoB)
                    prev.append(st)
                    if st >= 1:
                        emit_proj(prev.pop(0))
                for st in prev:
                    emit_proj(st)

    nc.compile()
    return nc


def make_in_maps(x, W_attn, b_attn, W_proj, b_proj):
    bf16 = ml_dtypes.bfloat16
    x = np.asarray(x, dtype=np.float32)
    W_attn = np.asarray(W_attn, dtype=np.float32)
    b_attn = np.asarray(b_attn, dtype=np.float32)
    W_proj = np.asarray(W_proj, dtype=np.float32)
    in_maps = []
    for i in range(NCORES):
        b, g = i // 2, i % 2
        j0 = g * JL
        in_maps.append({
            "xT": np.ascontiguousarray(x[b].T).astype(bf16),
            "wq": W_attn[:, j0:j0 + JL].astype(bf16),
            "wk": W_attn[:, E + j0:E + j0 + JL].astype(bf16),
            "wv": W_attn[:, 2 * E + j0:2 * E + j0 + JL].astype(bf16),
            "wp": W_proj[j0:j0 + JL, :].astype(bf16),
            "bq": np.ascontiguousarray(
                b_attn[j0:j0 + JL].astype(np.float32).reshape(JT, P).T),
            "bk": np.ascontiguousarray(
                b_attn[E + j0:E + j0 + JL].astype(np.float32).reshape(JT, P).T),
        })
    return in_maps


def kernel(x, W_attn, b_attn, W_proj, b_proj):
    global _NC_CACHE
    x = np.asarray(x, dtype=np.float32)
    W_attn = np.asarray(W_attn, dtype=np.float32)
    b_attn = np.asarray(b_attn, dtype=np.float32)
    W_proj = np.asarray(W_proj, dtype=np.float32)
    b_proj = np.asarray(b_proj, dtype=np.float32)

    if _NC_CACHE is None:
        _NC_CACHE = build_nc()
    nc = _NC_CACHE

    in_maps = make_in_maps(x, W_attn, b_attn, W_proj, b_proj)
    res = run_bass_kernel_spmd(nc, in_maps, core_ids=list(range(NCORES)))

    # host unshard: sum the two head-group partials + exact bias corrections
    bias_row = b_proj.copy()
    for g in range(2):
        j0 = g * JL
        bv = b_attn[2 * E + j0:2 * E + j0 + JL].astype(np.float32)
        bias_row += bv @ W_proj[j0:j0 + JL, :].astype(np.float32)

    full = np.empty((B, S, E), np.float32)
    for b in range(B):
        full[b] = (res.results[2 * b]["out"] + res.results[2 * b + 1]["out"]
                   + bias_row[None, :])
    return full


# revision 37
# speedup vs baseline: 1.3141x; 1.1278x over previous
"""Causal multi-head attention block (B=4, S=1024, E=1024, H=16, D=64) on 8 TRN2 cores.

Sharding: data-parallel over batch (4) x tensor-parallel over heads (2 groups of 8).
Core i handles batch i//2, head-group i%2. Each core computes its partial output
projection (row-parallel W_proj); the host sums the two TP partials per batch and
applies the (exact) bias corrections.

Schedule (v2): PE-density-first.
  - Startup: xT streams per-ktile on the sync HWDGE queue while (wq,wk) kt-pairs
    stream on the scalar queue; a kt-major 4-chain group {q0,k0,q1,k1} accumulates
    in all 8 PSUM banks (scoped pool) so the PE saturates as tiles land.
  - Attention: per head-pair p, QK^T chunks ([sk,sq] blocks via row-half pairing,
    concurrent in the two 64-row groups) feed ScalarE exp chunks into a PACKED
    staircase pT buffer (only the causal-valid blocks); diagonal masks are small
    per-block gpsimd multiplies right after their exp chunk. Chain groups q2k2,
    q3k3 and the v chains interleave as PE filler between QK chunks so the exp
    stream never gates the PE.
  - PV: lhsT=v_aug (ones column -> denominator row); normalize = gpsimd
    partition-broadcast of the PSUM denom row + DVE reciprocal + multiply.
  - Tail: pair 3's PV runs per-128-col sq chunks, interleaved with the 8 output
    projection tiles (ACT evacuates PSUM, per-st DMA out on the sync queue).
Host: out[b] = out_partial[2b] + out_partial[2b+1] + (bv_0 Wp_0 + bv_1 Wp_1 + b_proj)
(the v-bias term is exact because softmax rows sum to 1).
"""

import numpy as np
import ml_dtypes

import concourse.bass as bass
import concourse.tile as tile
from concourse import bacc, mybir
from concourse.bass_utils import run_bass_kernel_spmd
from concourse.masks import make_identity, make_lower_triangular

BF16 = mybir.dt.bfloat16
F32 = mybir.dt.float32

B, S, E = 4, 1024, 1024
H_TOT, D = 16, 64
NCORES = 8
HL = 8            # heads per core
JL = HL * D       # 512 local qkv dim
P = 128
ET = E // P       # 8 k-tiles over embed dim
JT = JL // P      # 4 partition-tiles over local qkv dim

# packed staircase offsets: for sk-tile t, columns sq in [t*P, S) are stored at
# pt[:, POFF[t] + (sq - t*P)]
POFF = [t * S - P * (t * (t - 1) // 2) for t in range(ET)]
PTW = POFF[ET - 1] + (S - (ET - 1) * P)   # 4608 packed cols per head

_NC_CACHE = None


def build_nc():
    nc = bacc.Bacc()

    xT = nc.declare_dram_parameter("xT", [E, S], BF16, isOutput=False)
    wq = nc.declare_dram_parameter("wq", [E, JL], BF16, isOutput=False)
    wk = nc.declare_dram_parameter("wk", [E, JL], BF16, isOutput=False)
    wv = nc.declare_dram_parameter("wv", [E, JL], BF16, isOutput=False)
    wp = nc.declare_dram_parameter("wp", [JL, E], BF16, isOutput=False)
    bq = nc.declare_dram_parameter("bq", [P, JT], F32, isOutput=False)
    bk = nc.declare_dram_parameter("bk", [P, JT], F32, isOutput=False)
    out = nc.declare_dram_parameter("out", [S, E], F32, isOutput=True)

    with tile.TileContext(nc) as tc:
        with (
            tc.tile_pool(name="singles", bufs=1) as singles,
            tc.tile_pool(name="pt", bufs=8) as pt_pool,
            tc.tile_pool(name="bc", bufs=3) as bc_pool,
            tc.tile_pool(name="den", bufs=3) as den_pool,
            tc.tile_pool(name="outst", bufs=2) as out_pool,
        ):
            # ---- static input tiles ----
            xT_sb = singles.tile([P, ET, S], BF16)
            wq_sb = singles.tile([P, ET, JL], BF16)
            wk_sb = singles.tile([P, ET, JL], BF16)
            wv_sb = singles.tile([P, ET, JL], BF16)
            wp_sb = singles.tile([P, JT, E], BF16)
            bq_sb = singles.tile([P, JT], F32)
            bk_sb = singles.tile([P, JT], F32)
            xT_r = xT[:, :].rearrange("(o p) s -> p o s", p=P)
            wq_r = wq[:, :].rearrange("(o p) j -> p o j", p=P)
            wk_r = wk[:, :].rearrange("(o p) j -> p o j", p=P)
            wv_r = wv[:, :].rearrange("(o p) j -> p o j", p=P)
            wp_r = wp[:, :].rearrange("(o p) e -> p o e", p=P)

            # biases on the gpsimd (SWDGE) queue: tiny, cheap trigger
            nc.gpsimd.dma_start(out=bq_sb[:], in_=bq[:, :])
            nc.gpsimd.dma_start(out=bk_sb[:], in_=bk[:, :])
            # (wq,wk) kt-pairs on the scalar HWDGE queue, interleaved so the
            # startup chain group's kt-steps unlock in arrival order
            for c in range(0, ET, 2):
                nc.scalar.dma_start(out=wq_sb[:, c:c + 2], in_=wq_r[:, c:c + 2])
                nc.scalar.dma_start(out=wk_sb[:, c:c + 2], in_=wk_r[:, c:c + 2])
            # xT per-ktile on the sync queue; wv and wp queue AFTER x so the
            # critical x/wq/wk set gets the full DMA bandwidth first
            for kt in range(ET):
                nc.sync.dma_start(out=xT_sb[:, kt], in_=xT_r[:, kt])
            for c in range(0, ET, 2):
                nc.sync.dma_start(out=wv_sb[:, c:c + 2], in_=wv_r[:, c:c + 2])
            nc.sync.dma_start(out=wp_sb[:, 0:2], in_=wp_r[:, 0:2])
            nc.sync.dma_start(out=wp_sb[:, 2:4], in_=wp_r[:, 2:4])

            # additive causal mask for diagonal logit blocks: one PE matmul
            # accumulates -2000 onto the sq<sk half of each diagonal psl block
            # (exp then emits exact zeros) -> no post-exp mask op at all.
            # negC[sk, sq] = -2000 where sq < sk else 0; ident = I_128.
            negC_sb = singles.tile([P, P], BF16)
            make_lower_triangular(nc, negC_sb[:], val=-2000.0, diag=False)
            ident_sb = singles.tile([P, P], BF16)
            make_identity(nc, ident_sb[:])

            qT_sb = singles.tile([P, JT, S], BF16)   # row j = h*64+d, head-major
            kT_sb = singles.tile([P, JT, S], BF16)
            o2T_sb = singles.tile([P, JT, S], BF16)  # normalized attn out
            vaug_sb = singles.tile([P, ET, HL, D + 1], BF16)  # [sk_p, sk_tile, head, d|ones]
            nc.vector.memset(vaug_sb[:, :, :, D:D + 1], 1.0)

            # ---------- chain groups (q/k projections), kt-major ----------
            def chain_group(jts, pool, split=False):
                """Returns (step, finish): step(kt) emits the 4*len(jts) matmuls
                for that ktile; finish(jt) the bias-adds (q on DVE, k on gpsimd
                so the two biases drain in parallel)."""
                accs = {}
                for jt in jts:
                    for tag in ("q", "k"):
                        for nb in range(2):
                            accs[(tag, jt, nb)] = pool.tile(
                                [P, 512], F32, tag="mm", name=f"c{tag}{jt}{nb}")

                def step(kt):
                    for jt in jts:
                        for tag, w_sb in (("q", wq_sb), ("k", wk_sb)):
                            for nb in range(2):
                                nc.tensor.matmul(
                                    accs[(tag, jt, nb)][:],
                                    lhsT=w_sb[:, kt, jt * P:(jt + 1) * P],
                                    rhs=xT_sb[:, kt, nb * 512:(nb + 1) * 512],
                                    start=(kt == 0), stop=(kt == ET - 1),
                                )

                def finish(jt):
                    # q-bias on DVE, k-bias on ACT (idle pre-exp) in parallel
                    for nb in range(2):
                        nc.vector.tensor_scalar_add(
                            qT_sb[:, jt, nb * 512:(nb + 1) * 512],
                            accs[("q", jt, nb)][:], bq_sb[:, jt:jt + 1],
                        )
                    for nb in range(2):
                        nc.scalar.activation(
                            out=kT_sb[:, jt, nb * 512:(nb + 1) * 512],
                            in_=accs[("k", jt, nb)][:],
                            func=mybir.ActivationFunctionType.Identity,
                            bias=bk_sb[:, jt:jt + 1],
                        )

                return step, finish

            # ---------- HAM warm-up: dummy matmuls during the DMA-wait window
            # (PE runs at 1.2 GHz until ~3.4us of continuous activity) ----------
            warm_sb = singles.tile([P, 512], BF16)
            nc.vector.memset(warm_sb[:], 0.5)
            with tc.tile_pool(name="ps_w", bufs=1, space="PSUM") as ps_w:
                wps = ps_w.tile([P, 512], F32, tag="w", name="warm")
                NWARM = 12
                for i in range(NWARM):
                    nc.tensor.matmul(
                        wps[:], lhsT=warm_sb[:, 0:P], rhs=warm_sb[:],
                        start=(i == 0), stop=(i == NWARM - 1),
                    )
                nc.vector.tensor_copy(out=warm_sb[:, 0:1], in_=wps[:, 0:1])

            # ---------- startup: 4-chain kt-major group in 8 PSUM banks ----------
            with tc.tile_pool(name="ps_g0", bufs=8, space="PSUM") as ps_g0:
                g0_step, g0_finish = chain_group([0, 1], ps_g0)
                for kt in range(ET):
                    g0_step(kt)
                g0_finish(0)
                g0_finish(1)

            with (
                tc.tile_pool(name="ps_mm", bufs=2, space="PSUM") as ps_mm,
                tc.tile_pool(name="ps_l", bufs=2, space="PSUM") as ps_l,
                tc.tile_pool(name="ps_o", bufs=2, space="PSUM") as ps_o,
            ):
                pts = {}

                def emit_chain(jt, tag):
                    """one q- or k-projection chain (2 PSUM slots, serial kt)"""
                    w_sb, b_sb, dst = ((wq_sb, bq_sb, qT_sb) if tag == "q"
                                       else (wk_sb, bk_sb, kT_sb))
                    accs = [ps_mm.tile([P, 512], F32, tag="mm", name=f"c{tag}{jt}{nb}")
                            for nb in range(2)]
                    for kt in range(ET):
                        for nb in range(2):
                            nc.tensor.matmul(
                                accs[nb][:],
                                lhsT=w_sb[:, kt, jt * P:(jt + 1) * P],
                                rhs=xT_sb[:, kt, nb * 512:(nb + 1) * 512],
                                start=(kt == 0), stop=(kt == ET - 1),
                            )
                    for nb in range(2):
                        nc.vector.tensor_scalar_add(
                            dst[:, jt, nb * 512:(nb + 1) * 512],
                            accs[nb][:], b_sb[:, jt:jt + 1],
                        )

                def emit_v_chain(st):
                    ps = ps_mm.tile([P, 512], F32, tag="mm", name=f"v{st}")
                    for kt in range(ET):
                        nc.tensor.matmul(
                            ps[:],
                            lhsT=xT_sb[:, kt, st * P:(st + 1) * P],
                            rhs=wv_sb[:, kt, :],
                            start=(kt == 0), stop=(kt == ET - 1),
                        )
                    nc.vector.tensor_copy(
                        out=vaug_sb[:, st, :, 0:D],
                        in_=ps[:].rearrange("p (h d) -> p h d", h=HL),
                    )

                def emit_pair(p, fillers):
                    """QK^T + exp + per-block diag mask for heads (2p, 2p+1).
                    fillers: list of closures, spread across the t loop."""
                    views = []
                    for hh in (2 * p, 2 * p + 1):
                        jt0, po = hh // 2, (hh % 2) * 64
                        pT = pt_pool.tile([P, PTW], BF16, tag="pt", name=f"pt{hh}")
                        pts[hh] = pT
                        views.append((qT_sb[po:po + 64, jt0, :],
                                      kT_sb[po:po + 64, jt0, :], pT))
                    fi, nf = 0, len(fillers)
                    for t in range(ET):
                        lo = t * P
                        # psl chunks at ABSOLUTE sq offsets: each <=512-col
                        # matmul stays inside one 2KB PSUM bank
                        psls = [ps_l.tile([P, 1024], F32, tag="psl",
                                          name=f"psl{p}_{t}_{vi}")
                                for vi in range(len(views))]
                        for cb in range(2):
                            c0, c1 = cb * 512, (cb + 1) * 512
                            s0 = max(lo, c0)
                            if s0 >= c1:
                                continue
                            # the two heads' K=64 matmuls sit in disjoint row
                            # halves -> run concurrently on the array
                            for (qh, kh, _pT), psl in zip(views, psls):
                                nc.tensor.matmul(
                                    psl[:, s0:c1],
                                    lhsT=kh[:, lo:lo + P],
                                    rhs=qh[:, s0:c1],
                                    start=True, stop=True,
                                )
                        # accumulate the additive causal mask onto the diagonal
                        # block, then exp the whole row stripe
                        for (_qh, _kh, pT), psl in zip(views, psls):
                            nc.tensor.matmul(
                                psl[:, lo:lo + P],
                                lhsT=ident_sb[:],
                                rhs=negC_sb[:],
                                start=False, stop=True,
                                skip_group_check=True,
                            )
                            nc.scalar.activation(
                                out=pT[:, POFF[t]:POFF[t] + S - lo],
                                in_=psl[:, lo:S],
                                func=mybir.ActivationFunctionType.Exp,
                                scale=0.125,
                            )
                        # interleave filler PE work to hide the exp latency
                        want = (t + 1) * nf // ET
                        while fi < want:
                            fillers[fi]()
                            fi += 1
                    while fi < nf:
                        fillers[fi]()
                        fi += 1

                def norm_chunk(h, pso, r0, r1, c0):
                    """o2T[head rows, c0:c0+(r1-r0)] = pso[:64, r0:r1] / pso[64, r0:r1]"""
                    jt0, po = h // 2, (h % 2) * 64
                    w = r1 - r0
                    den = den_pool.tile([1, 512], F32, tag="den", name=f"dn{h}_{c0}")
                    # custom-DVE reciprocal needs SBUF input at partition 0: stage first
                    nc.vector.tensor_copy(out=den[:, 0:w], in_=pso[D:D + 1, r0:r1])
                    nc.vector.reciprocal_approx_fast(out=den[:, 0:w], in_=den[:, 0:w])
                    bcst = bc_pool.tile([64, 512], F32, tag="bc", name=f"bc{h}_{c0}")
                    nc.gpsimd.partition_broadcast(bcst[:, 0:w], den[:, 0:w])
                    nc.vector.tensor_mul(
                        out=o2T_sb[po:po + 64, jt0, c0:c0 + w],
                        in0=pso[:D, r0:r1], in1=bcst[:, 0:w],
                    )

                def emit_pv_sqb(h, sqb, pool=None):
                    """PV + normalize for head h, one 512-col sq half"""
                    pT = pts[h]
                    c0, c1 = sqb * 512, (sqb + 1) * 512
                    pl = pool if pool is not None else ps_o
                    tg = "pso" if pl is ps_o else "mm"
                    pso = pl.tile([P, 512], F32, tag=tg, name=f"pso{h}_{sqb}")
                    ts = [t for t in range(ET) if t * P < c1]
                    for i, t in enumerate(ts):
                        s0 = max(t * P, c0)
                        nc.tensor.matmul(
                            pso[:D + 1, s0 - c0:512],
                            lhsT=vaug_sb[:, t, h, :],
                            rhs=pT[:, POFF[t] + s0 - t * P:POFF[t] + c1 - t * P],
                            start=(i == 0), stop=(i == len(ts) - 1),
                            skip_group_check=True,
                        )
                    norm_chunk(h, pso, 0, 512, c0)

                def emit_pv_fine(h, st, pso):
                    """PV + normalize for head h, one 128-col sq chunk st"""
                    pT = pts[h]
                    c0, r0 = st * P, (st % 4) * P
                    for t in range(st + 1):
                        nc.tensor.matmul(
                            pso[:D + 1, r0:r0 + P],
                            lhsT=vaug_sb[:, t, h, :],
                            rhs=pT[:, POFF[t] + (st - t) * P:POFF[t] + (st - t + 1) * P],
                            start=(t == 0), stop=(t == st),
                            skip_group_check=True,
                        )
                    norm_chunk(h, pso, r0, r0 + P, c0)

                def emit_proj(st):
                    # [P,1024] accumulator in the (drained) wide psl slots:
                    # eb halves live in separate banks, single ACT evacuation
                    psf = ps_l.tile([P, 1024], F32, tag="psl", name=f"pj{st}")
                    for kt in range(JT):
                        for eb in range(2):
                            nc.tensor.matmul(
                                psf[:, eb * 512:(eb + 1) * 512],
                                lhsT=o2T_sb[:, kt, st * P:(st + 1) * P],
                                rhs=wp_sb[:, kt, eb * 512:(eb + 1) * 512],
                                start=(kt == 0), stop=(kt == JT - 1),
                                skip_group_check=True,
                            )
                    ob = out_pool.tile([P, 1024], F32)
                    nc.scalar.copy(out=ob[:], in_=psf[:])
                    nc.sync.dma_start(out=out[st * P:(st + 1) * P, :], in_=ob[:])

                # ---------- master schedule ----------
                # PV sq-halves ride inside the (exp-paced) pair slots as PE
                # filler; their norm chains overlap the exp stream.
                emit_pair(0, [lambda: emit_chain(2, "q"), lambda: emit_chain(2, "k")])
                emit_pair(1, [lambda: emit_chain(3, "q"), lambda: emit_chain(3, "k"),
                              lambda: emit_v_chain(0), lambda: emit_v_chain(1)])
                emit_pair(2, [lambda st=st: emit_v_chain(st) for st in (2, 3, 4)]
                          + [lambda: emit_pv_sqb(0, 0), lambda: emit_pv_sqb(1, 0)])
                emit_pair(3, [lambda st=st: emit_v_chain(st) for st in (5, 6, 7)]
                          + [lambda: emit_pv_sqb(2, 0), lambda: emit_pv_sqb(3, 0),
                             lambda: emit_pv_sqb(0, 1), lambda: emit_pv_sqb(1, 1)])

                # alternate PSUM pools -> 4 norm chains in flight
                for i, (h, sqb) in enumerate(
                        ((4, 0), (5, 0), (2, 1), (3, 1), (4, 1), (5, 1))):
                    emit_pv_sqb(h, sqb, pool=(ps_mm if i % 2 else ps_o))

                # pair 3: per-128-col PV chunks (fine psos in the drained
                # chain banks) interleaved with the output projection tiles
                psoA = ps_mm.tile([P, 512], F32, tag="mm", name="psoA_lo")
                psoB = ps_mm.tile([P, 512], F32, tag="mm", name="psoB_lo")
                prev = []
                for st in range(ET):
                    if st == 4:
                        psoA = ps_mm.tile([P, 512], F32, tag="mm", name="psoA_hi")
                        psoB = ps_mm.tile([P, 512], F32, tag="mm", name="psoB_hi")
                    emit_pv_fine(6, st, psoA)
                    emit_pv_fine(7, st, psoB)
                    prev.append(st)
                    if st >= 1:
                        emit_proj(prev.pop(0))
                for st in prev:
                    emit_proj(st)

    nc.compile()
    return nc


def make_in_maps(x, W_attn, b_attn, W_proj, b_proj):
    bf16 = ml_dtypes.bfloat16
    x = np.asarray(x, dtype=np.float32)
    W_attn = np.asarray(W_attn, dtype=np.float32)
    b_attn = np.asarray(b_attn, dtype=np.float32)
    W_proj = np.asarray(W_proj, dtype=np.float32)
    in_maps = []
    for i in range(NCORES):
        b, g = i // 2, i % 2
        j0 = g * JL
        in_maps.append({
            "xT": np.ascontiguousarray(x[b].T).astype(bf16),
            "wq": W_attn[:, j0:j0 + JL].astype(bf16),
            "wk": W_attn[:, E + j0:E + j0 + JL].astype(bf16),
            "wv": W_attn[:, 2 * E + j0:2 * E + j0 + JL].astype(bf16),
            "wp": W_proj[j0:j0 + JL, :].astype(bf16),
            "bq": np.ascontiguousarray(
                b_attn[j0:j0 + JL].astype(np.float32).reshape(JT, P).T),
            "bk": np.ascontiguousarray(
                b_attn[E + j0:E + j0 + JL].astype(np.float32).reshape(JT, P).T),
        })
    return in_maps


def kernel(x, W_attn, b_attn, W_proj, b_proj):
    global _NC_CACHE
    x = np.asarray(x, dtype=np.float32)
    W_attn = np.asarray(W_attn, dtype=np.float32)
    b_attn = np.asarray(b_attn, dtype=np.float32)
    W_proj = np.asarray(W_proj, dtype=np.float32)
    b_proj = np.asarray(b_proj, dtype=np.float32)

    if _NC_CACHE is None:
        _NC_CACHE = build_nc()
    nc = _NC_CACHE

    in_maps = make_in_maps(x, W_attn, b_attn, W_proj, b_proj)
    res = run_bass_kernel_spmd(nc, in_maps, core_ids=list(range(NCORES)))

    # host unshard: sum the two head-group partials + exact bias corrections
    bias_row = b_proj.copy()
    for g in range(2):
        j0 = g * JL
        bv = b_attn[2 * E + j0:2 * E + j0 + JL].astype(np.float32)
        bias_row += bv @ W_proj[j0:j0 + JL, :].astype(np.float32)

    full = np.empty((B, S, E), np.float32)
    for b in range(B):
        full[b] = (res.results[2 * b]["out"] + res.results[2 * b + 1]["out"]
                   + bias_row[None, :])
    return full


# revision 42
# speedup vs baseline: 1.5262x; 1.1614x over previous
"""Causal multi-head attention block (B=4, S=1024, E=1024, H=16, D=64) on 8 TRN2 cores.

Sharding: data-parallel over batch (4) x tensor-parallel over heads (2 groups of 8).
Core i handles batch i//2, head-group i%2. Each core computes its partial output
projection (row-parallel W_proj); the host sums the two TP partials per batch and
applies the (exact) bias corrections.

Schedule (v2): PE-density-first.
  - Startup: xT streams per-ktile on the sync HWDGE queue while (wq,wk) kt-pairs
    stream on the scalar queue; a kt-major 4-chain group {q0,k0,q1,k1} accumulates
    in all 8 PSUM banks (scoped pool) so the PE saturates as tiles land.
  - Attention: per head-pair p, QK^T chunks ([sk,sq] blocks via row-half pairing,
    concurrent in the two 64-row groups) feed ScalarE exp chunks into a PACKED
    staircase pT buffer (only the causal-valid blocks); diagonal masks are small
    per-block gpsimd multiplies right after their exp chunk. Chain groups q2k2,
    q3k3 and the v chains interleave as PE filler between QK chunks so the exp
    stream never gates the PE.
  - PV: lhsT=v_aug (ones column -> denominator row); normalize = gpsimd
    partition-broadcast of the PSUM denom row + DVE reciprocal + multiply.
  - Tail: pair 3's PV runs per-128-col sq chunks, interleaved with the 8 output
    projection tiles (ACT evacuates PSUM, per-st DMA out on the sync queue).
Host: out[b] = out_partial[2b] + out_partial[2b+1] + (bv_0 Wp_0 + bv_1 Wp_1 + b_proj)
(the v-bias term is exact because softmax rows sum to 1).
"""

import numpy as np
import ml_dtypes

import concourse.bass as bass
import concourse.tile as tile
from concourse import bacc, mybir
from concourse.bass_utils import run_bass_kernel_spmd
from concourse.masks import make_identity, make_lower_triangular

BF16 = mybir.dt.bfloat16
F32 = mybir.dt.float32

B, S, E = 4, 1024, 1024
H_TOT, D = 16, 64
NCORES = 8
HL = 8            # heads per core
JL = HL * D       # 512 local qkv dim
P = 128
ET = E // P       # 8 k-tiles over embed dim
JT = JL // P      # 4 partition-tiles over local qkv dim

# packed staircase offsets: for sk-tile t, columns sq in [t*P, S) are stored at
# pt[:, POFF[t] + (sq - t*P)]
POFF = [t * S - P * (t * (t - 1) // 2) for t in range(ET)]
PTW = POFF[ET - 1] + (S - (ET - 1) * P)   # 4608 packed cols per head

_NC_CACHE = None


def build_nc():
    nc = bacc.Bacc()

    xT = nc.declare_dram_parameter("xT", [E, S], BF16, isOutput=False)
    wq = nc.declare_dram_parameter("wq", [E, JL], BF16, isOutput=False)
    wk = nc.declare_dram_parameter("wk", [E, JL], BF16, isOutput=False)
    wv = nc.declare_dram_parameter("wv", [E, JL], BF16, isOutput=False)
    wp = nc.declare_dram_parameter("wp", [JL, E], BF16, isOutput=False)
    bq = nc.declare_dram_parameter("bq", [P, JT], F32, isOutput=False)
    bk = nc.declare_dram_parameter("bk", [P, JT], F32, isOutput=False)
    out = nc.declare_dram_parameter("out", [S, E], F32, isOutput=True)

    with tile.TileContext(nc) as tc:
        with (
            tc.tile_pool(name="singles", bufs=1) as singles,
            tc.tile_pool(name="pt", bufs=8) as pt_pool,
            tc.tile_pool(name="bc", bufs=3) as bc_pool,
            tc.tile_pool(name="den", bufs=3) as den_pool,
            tc.tile_pool(name="outst", bufs=2) as out_pool,
        ):
            # ---- static input tiles ----
            xT_sb = singles.tile([P, ET, S], BF16)
            wq_sb = singles.tile([P, ET, JL], BF16)
            wk_sb = singles.tile([P, ET, JL], BF16)
            wv_sb = singles.tile([P, ET, JL], BF16)
            wp_sb = singles.tile([P, JT, E], BF16)
            bq_sb = singles.tile([P, JT], F32)
            bk_sb = singles.tile([P, JT], F32)
            xT_r = xT[:, :].rearrange("(o p) s -> p o s", p=P)
            wq_r = wq[:, :].rearrange("(o p) j -> p o j", p=P)
            wk_r = wk[:, :].rearrange("(o p) j -> p o j", p=P)
            wv_r = wv[:, :].rearrange("(o p) j -> p o j", p=P)
            wp_r = wp[:, :].rearrange("(o p) e -> p o e", p=P)

            # biases on the gpsimd (SWDGE) queue: tiny, cheap trigger
            nc.gpsimd.dma_start(out=bq_sb[:], in_=bq[:, :])
            nc.gpsimd.dma_start(out=bk_sb[:], in_=bk[:, :])
            # (wq,wk) kt-pairs on the scalar HWDGE queue, interleaved so the
            # startup chain group's kt-steps unlock in arrival order
            for c in range(0, ET, 2):
                nc.scalar.dma_start(out=wq_sb[:, c:c + 2], in_=wq_r[:, c:c + 2])
                nc.scalar.dma_start(out=wk_sb[:, c:c + 2], in_=wk_r[:, c:c + 2])
            # xT per-ktile on the sync queue; wv and wp queue AFTER x so the
            # critical x/wq/wk set gets the full DMA bandwidth first
            for kt in range(ET):
                nc.sync.dma_start(out=xT_sb[:, kt], in_=xT_r[:, kt])
            for c in range(0, ET, 2):
                nc.sync.dma_start(out=wv_sb[:, c:c + 2], in_=wv_r[:, c:c + 2])
            nc.sync.dma_start(out=wp_sb[:, 0:2], in_=wp_r[:, 0:2])
            nc.sync.dma_start(out=wp_sb[:, 2:4], in_=wp_r[:, 2:4])

            # additive causal mask for diagonal logit blocks: one PE matmul
            # accumulates -2000 onto the sq<sk half of each diagonal psl block
            # (exp then emits exact zeros) -> no post-exp mask op at all.
            # negC[sk, sq] = -2000 where sq < sk else 0; ident = I_128.
            negC_sb = singles.tile([P, P], BF16)
            make_lower_triangular(nc, negC_sb[:], val=-2000.0, diag=False)
            ident_sb = singles.tile([P, P], BF16)
            make_identity(nc, ident_sb[:])

            qT_sb = singles.tile([P, JT, S], BF16)   # row j = h*64+d, head-major
            kT_sb = singles.tile([P, JT, S], BF16)
            o2T_sb = singles.tile([P, JT, S], BF16)  # normalized attn out
            vaug_sb = singles.tile([P, ET, HL, D + 1], BF16)  # [sk_p, sk_tile, head, d|ones]
            nc.vector.memset(vaug_sb[:, :, :, D:D + 1], 1.0)

            # ---------- chain groups (q/k projections), kt-major ----------
            def chain_group(jts, pool, split=False):
                """Returns (step, finish): step(kt) emits the 4*len(jts) matmuls
                for that ktile; finish(jt) the bias-adds (q on DVE, k on gpsimd
                so the two biases drain in parallel)."""
                accs = {}
                for jt in jts:
                    for tag in ("q", "k"):
                        for nb in range(2):
                            accs[(tag, jt, nb)] = pool.tile(
                                [P, 512], F32, tag="mm", name=f"c{tag}{jt}{nb}")

                def step(kt):
                    for jt in jts:
                        for tag, w_sb in (("q", wq_sb), ("k", wk_sb)):
                            for nb in range(2):
                                nc.tensor.matmul(
                                    accs[(tag, jt, nb)][:],
                                    lhsT=w_sb[:, kt, jt * P:(jt + 1) * P],
                                    rhs=xT_sb[:, kt, nb * 512:(nb + 1) * 512],
                                    start=(kt == 0), stop=(kt == ET - 1),
                                )

                def finish(jt):
                    # q-bias on DVE, k-bias on ACT (idle pre-exp) in parallel
                    for nb in range(2):
                        nc.vector.tensor_scalar_add(
                            qT_sb[:, jt, nb * 512:(nb + 1) * 512],
                            accs[("q", jt, nb)][:], bq_sb[:, jt:jt + 1],
                        )
                    for nb in range(2):
                        nc.scalar.activation(
                            out=kT_sb[:, jt, nb * 512:(nb + 1) * 512],
                            in_=accs[("k", jt, nb)][:],
                            func=mybir.ActivationFunctionType.Identity,
                            bias=bk_sb[:, jt:jt + 1],
                        )

                return step, finish

            # ---------- HAM warm-up: dummy matmuls during the DMA-wait window
            # (PE runs at 1.2 GHz until ~3.4us of continuous activity) ----------
            warm_sb = singles.tile([P, 512], BF16)
            nc.vector.memset(warm_sb[:], 0.5)
            with tc.tile_pool(name="ps_w", bufs=1, space="PSUM") as ps_w:
                wps = ps_w.tile([P, 512], F32, tag="w", name="warm")
                NWARM = 12
                for i in range(NWARM):
                    nc.tensor.matmul(
                        wps[:], lhsT=warm_sb[:, 0:P], rhs=warm_sb[:],
                        start=(i == 0), stop=(i == NWARM - 1),
                    )
                nc.vector.tensor_copy(out=warm_sb[:, 0:1], in_=wps[:, 0:1])

            # ---------- startup: 4-chain kt-major group in 8 PSUM banks ----------
            with tc.tile_pool(name="ps_g0", bufs=8, space="PSUM") as ps_g0:
                g0_step, g0_finish = chain_group([0, 1], ps_g0)
                for kt in range(ET):
                    g0_step(kt)
                g0_finish(0)
                g0_finish(1)

            with (
                tc.tile_pool(name="ps_mm", bufs=2, space="PSUM") as ps_mm,
                tc.tile_pool(name="ps_l", bufs=2, space="PSUM") as ps_l,
                tc.tile_pool(name="ps_o", bufs=2, space="PSUM") as ps_o,
            ):
                pts = {}

                def chain_steps(jt, tag):
                    """q/k-projection chain as a list of small filler steps
                    (2 matmuls each) so the exp stream is never starved."""
                    w_sb, b_sb, dst = ((wq_sb, bq_sb, qT_sb) if tag == "q"
                                       else (wk_sb, bk_sb, kT_sb))
                    state = {}

                    def step(kt):
                        if kt == 0:
                            state["a"] = [
                                ps_mm.tile([P, 512], F32, tag="mm",
                                           name=f"c{tag}{jt}{nb}")
                                for nb in range(2)]
                        for nb in range(2):
                            nc.tensor.matmul(
                                state["a"][nb][:],
                                lhsT=w_sb[:, kt, jt * P:(jt + 1) * P],
                                rhs=xT_sb[:, kt, nb * 512:(nb + 1) * 512],
                                start=(kt == 0), stop=(kt == ET - 1),
                            )
                        if kt == ET - 1:
                            for nb in range(2):
                                nc.vector.tensor_scalar_add(
                                    dst[:, jt, nb * 512:(nb + 1) * 512],
                                    state["a"][nb][:], b_sb[:, jt:jt + 1],
                                )

                    return [lambda kt=kt: step(kt) for kt in range(ET)]

                def v_steps(st):
                    """v-projection chain for sk-tile st, as 4 filler steps"""
                    state = {}

                    def step(k0):
                        if k0 == 0:
                            state["a"] = ps_mm.tile([P, 512], F32, tag="mm",
                                                    name=f"v{st}")
                        for kt in (k0, k0 + 1):
                            nc.tensor.matmul(
                                state["a"][:],
                                lhsT=xT_sb[:, kt, st * P:(st + 1) * P],
                                rhs=wv_sb[:, kt, :],
                                start=(kt == 0), stop=(kt == ET - 1),
                            )
                        if k0 == ET - 2:
                            nc.vector.tensor_copy(
                                out=vaug_sb[:, st, :, 0:D],
                                in_=state["a"][:].rearrange("p (h d) -> p h d", h=HL),
                            )

                    return [lambda k0=k0: step(k0) for k0 in range(0, ET, 2)]

                def emit_pair(p, fillers):
                    """QK^T + exp + per-block diag mask for heads (2p, 2p+1).
                    fillers: list of closures, spread across the t loop."""
                    views = []
                    for hh in (2 * p, 2 * p + 1):
                        jt0, po = hh // 2, (hh % 2) * 64
                        pT = pt_pool.tile([P, PTW], BF16, tag="pt", name=f"pt{hh}")
                        pts[hh] = pT
                        views.append((qT_sb[po:po + 64, jt0, :],
                                      kT_sb[po:po + 64, jt0, :], pT))
                    fi, nf = 0, len(fillers)
                    for t in range(ET):
                        lo = t * P
                        # psl chunks at ABSOLUTE sq offsets: each <=512-col
                        # matmul stays inside one 2KB PSUM bank
                        psls = [ps_l.tile([P, 1024], F32, tag="psl",
                                          name=f"psl{p}_{t}_{vi}")
                                for vi in range(len(views))]
                        for cb in range(2):
                            c0, c1 = cb * 512, (cb + 1) * 512
                            s0 = max(lo, c0)
                            if s0 >= c1:
                                continue
                            # the two heads' K=64 matmuls sit in disjoint row
                            # halves -> run concurrently on the array
                            for (qh, kh, _pT), psl in zip(views, psls):
                                nc.tensor.matmul(
                                    psl[:, s0:c1],
                                    lhsT=kh[:, lo:lo + P],
                                    rhs=qh[:, s0:c1],
                                    start=True, stop=True,
                                )
                        # accumulate the additive causal mask onto the diagonal
                        # block, then exp the whole row stripe
                        for (_qh, _kh, pT), psl in zip(views, psls):
                            nc.tensor.matmul(
                                psl[:, lo:lo + P],
                                lhsT=ident_sb[:],
                                rhs=negC_sb[:],
                                start=False, stop=True,
                                skip_group_check=True,
                            )
                            nc.scalar.activation(
                                out=pT[:, POFF[t]:POFF[t] + S - lo],
                                in_=psl[:, lo:S],
                                func=mybir.ActivationFunctionType.Exp,
                                scale=0.125,
                            )
                        # interleave filler PE work to hide the exp latency
                        want = (t + 1) * nf // ET
                        while fi < want:
                            fillers[fi]()
                            fi += 1
                    while fi < nf:
                        fillers[fi]()
                        fi += 1

                def norm_chunk(h, pso, r0, r1, c0):
                    """o2T[head rows, c0:c0+(r1-r0)] = pso[:64, r0:r1] / pso[64, r0:r1]"""
                    jt0, po = h // 2, (h % 2) * 64
                    w = r1 - r0
                    den = den_pool.tile([1, 512], F32, tag="den", name=f"dn{h}_{c0}")
                    # custom-DVE reciprocal needs SBUF input at partition 0: stage first
                    nc.vector.tensor_copy(out=den[:, 0:w], in_=pso[D:D + 1, r0:r1])
                    nc.vector.reciprocal_approx_fast(out=den[:, 0:w], in_=den[:, 0:w])
                    bcst = bc_pool.tile([64, 512], F32, tag="bc", name=f"bc{h}_{c0}")
                    nc.gpsimd.partition_broadcast(bcst[:, 0:w], den[:, 0:w])
                    nc.vector.tensor_mul(
                        out=o2T_sb[po:po + 64, jt0, c0:c0 + w],
                        in0=pso[:D, r0:r1], in1=bcst[:, 0:w],
                    )

                def emit_pv_sqb(h, sqb, pool=None):
                    """PV + normalize for head h, one 512-col sq half"""
                    pT = pts[h]
                    c0, c1 = sqb * 512, (sqb + 1) * 512
                    pl = pool if pool is not None else ps_o
                    tg = "pso" if pl is ps_o else "mm"
                    pso = pl.tile([P, 512], F32, tag=tg, name=f"pso{h}_{sqb}")
                    ts = [t for t in range(ET) if t * P < c1]
                    for i, t in enumerate(ts):
                        s0 = max(t * P, c0)
                        nc.tensor.matmul(
                            pso[:D + 1, s0 - c0:512],
                            lhsT=vaug_sb[:, t, h, :],
                            rhs=pT[:, POFF[t] + s0 - t * P:POFF[t] + c1 - t * P],
                            start=(i == 0), stop=(i == len(ts) - 1),
                            skip_group_check=True,
                        )
                    norm_chunk(h, pso, 0, 512, c0)

                def pv_steps(h, sqb, pool=None):
                    """emit_pv_sqb split into ~2-matmul filler steps"""
                    pT = pts[h]
                    c0, c1 = sqb * 512, (sqb + 1) * 512
                    pl = pool if pool is not None else ps_o
                    tg = "pso" if pl is ps_o else "mm"
                    ts = [t for t in range(ET) if t * P < c1]
                    state = {}

                    def step(i0):
                        if i0 == 0:
                            state["a"] = pl.tile([P, 512], F32, tag=tg,
                                                 name=f"pso{h}_{sqb}")
                        pso = state["a"]
                        for i in range(i0, min(i0 + 2, len(ts))):
                            t = ts[i]
                            s0 = max(t * P, c0)
                            nc.tensor.matmul(
                                pso[:D + 1, s0 - c0:512],
                                lhsT=vaug_sb[:, t, h, :],
                                rhs=pT[:, POFF[t] + s0 - t * P:POFF[t] + c1 - t * P],
                                start=(i == 0), stop=(i == len(ts) - 1),
                                skip_group_check=True,
                            )
                        if i0 + 2 >= len(ts):
                            norm_chunk(h, pso, 0, 512, c0)

                    return [lambda i0=i0: step(i0) for i0 in range(0, len(ts), 2)]

                def emit_pv_fine(h, st, pso):
                    """PV + normalize for head h, one 128-col sq chunk st"""
                    pT = pts[h]
                    c0, r0 = st * P, (st % 4) * P
                    for t in range(st + 1):
                        nc.tensor.matmul(
                            pso[:D + 1, r0:r0 + P],
                            lhsT=vaug_sb[:, t, h, :],
                            rhs=pT[:, POFF[t] + (st - t) * P:POFF[t] + (st - t + 1) * P],
                            start=(t == 0), stop=(t == st),
                            skip_group_check=True,
                        )
                    norm_chunk(h, pso, r0, r0 + P, c0)

                def emit_proj(st):
                    # [P,1024] accumulator in the (drained) wide psl slots:
                    # eb halves live in separate banks, single ACT evacuation
                    psf = ps_l.tile([P, 1024], F32, tag="psl", name=f"pj{st}")
                    for kt in range(JT):
                        for eb in range(2):
                            nc.tensor.matmul(
                                psf[:, eb * 512:(eb + 1) * 512],
                                lhsT=o2T_sb[:, kt, st * P:(st + 1) * P],
                                rhs=wp_sb[:, kt, eb * 512:(eb + 1) * 512],
                                start=(kt == 0), stop=(kt == JT - 1),
                                skip_group_check=True,
                            )
                    ob = out_pool.tile([P, 1024], F32)
                    nc.scalar.copy(out=ob[:], in_=psf[:])
                    nc.sync.dma_start(out=out[st * P:(st + 1) * P, :], in_=ob[:])

                # ---------- master schedule ----------
                # Fillers are fine-grained (~2 matmuls each) so the PE reaches
                # the next QK chunk quickly and the exp stream never starves.
                # PV sq-halves ride inside the (exp-paced) pair slots; their
                # norm chains overlap the exp stream.
                emit_pair(0, chain_steps(2, "q") + chain_steps(2, "k"))
                emit_pair(1, chain_steps(3, "q") + chain_steps(3, "k")
                          + v_steps(0) + v_steps(1))
                emit_pair(2, v_steps(2) + v_steps(3) + v_steps(4)
                          + pv_steps(0, 0) + pv_steps(1, 0))
                emit_pair(3, v_steps(5) + v_steps(6) + v_steps(7)
                          + pv_steps(2, 0) + pv_steps(3, 0)
                          + pv_steps(0, 1) + pv_steps(1, 1))

                # alternate PSUM pools -> 4 norm chains in flight
                for i, (h, sqb) in enumerate(
                        ((4, 0), (5, 0), (2, 1), (3, 1), (4, 1), (5, 1))):
                    emit_pv_sqb(h, sqb, pool=(ps_mm if i % 2 else ps_o))

                # pair 3: sq-half 0 as regular PV halves, then per-128-col
                # chunks for sq-half 1 interleaved with the projection tiles
                # (ordered so each proj lands ~2 norm-latencies after its data)
                emit_pv_sqb(6, 0, pool=ps_o)
                emit_pv_sqb(7, 0, pool=ps_mm)
                psoA = ps_mm.tile([P, 512], F32, tag="mm", name="psoA_hi")
                psoB = ps_o.tile([P, 512], F32, tag="pso", name="psoB_hi")
                for st in range(4, ET):
                    emit_pv_fine(6, st, psoA)
                    emit_pv_fine(7, st, psoB)
                    if st > 4:
                        emit_proj(st - 1)      # needs fine norms st-1
                    emit_proj(st - 4)          # needs pair-3 sqb0 norms
                emit_proj(7)

    nc.compile()
    return nc


def make_in_maps(x, W_attn, b_attn, W_proj, b_proj):
    bf16 = ml_dtypes.bfloat16
    x = np.asarray(x, dtype=np.float32)
    W_attn = np.asarray(W_attn, dtype=np.float32)
    b_attn = np.asarray(b_attn, dtype=np.float32)
    W_proj = np.asarray(W_proj, dtype=np.float32)
    in_maps = []
    for i in range(NCORES):
        b, g = i // 2, i % 2
        j0 = g * JL
        in_maps.append({
            "xT": np.ascontiguousarray(x[b].T).astype(bf16),
            "wq": W_attn[:, j0:j0 + JL].astype(bf16),
            "wk": W_attn[:, E + j0:E + j0 + JL].astype(bf16),
            "wv": W_attn[:, 2 * E + j0:2 * E + j0 + JL].astype(bf16),
            "wp": W_proj[j0:j0 + JL, :].astype(bf16),
            "bq": np.ascontiguousarray(
                b_attn[j0:j0 + JL].astype(np.float32).reshape(JT, P).T),
            "bk": np.ascontiguousarray(
                b_attn[E + j0:E + j0 + JL].astype(np.float32).reshape(JT, P).T),
        })
    return in_maps


def kernel(x, W_attn, b_attn, W_proj, b_proj):
    global _NC_CACHE
    x = np.asarray(x, dtype=np.float32)
    W_attn = np.asarray(W_attn, dtype=np.float32)
    b_attn = np.asarray(b_attn, dtype=np.float32)
    W_proj = np.asarray(W_proj, dtype=np.float32)
    b_proj = np.asarray(b_proj, dtype=np.float32)

    if _NC_CACHE is None:
        _NC_CACHE = build_nc()
    nc = _NC_CACHE

    in_maps = make_in_maps(x, W_attn, b_attn, W_proj, b_proj)
    res = run_bass_kernel_spmd(nc, in_maps, core_ids=list(range(NCORES)))

    # host unshard: sum the two head-group partials + exact bias corrections
    bias_row = b_proj.copy()
    for g in range(2):
        j0 = g * JL
        bv = b_attn[2 * E + j0:2 * E + j0 + JL].astype(np.float32)
        bias_row += bv @ W_proj[j0:j0 + JL, :].astype(np.float32)

    full = np.empty((B, S, E), np.float32)
    for b in range(B):
        full[b] = (res.results[2 * b]["out"] + res.results[2 * b + 1]["out"]
                   + bias_row[None, :])
    return full
